# revision 1
# baseline (speedup 1.0000x reference)
"""Trainium2 Bass kernel for an 8-expert top-2 MoE layer.

Strategy (expert-parallel, per the sharding hint "all-to-all tokens by
top-k assignment"): the host computes the (tiny) gating matmul + softmax
+ top-2 routing, gathers each expert's assigned tokens, and ships one
expert per NeuronCore. Each core runs the heavy 2-layer MLP for its
expert over its assigned tokens (f32r matmuls on the PE array), applies
the gate weights on-device, and the host scatter-adds the two expert
contributions per token back together.

The MLP math runs fully transposed (tokens on the free dim) so that
 - W1/W2 slices feed the PE as stationary operands with no transposes,
 - the b1 bias + relu and (y + b2) * gate evictions are single fused
   DVE ops with per-partition scalars,
 - the per-token gate row is broadcast across partitions with one K=1
   matmul (ones[1,128]^T @ g[1,N] -> G[128,N]).

DMA-dispatch overhead (~0.6us per dma_start, serialized on the DGE
queue) is first-order here, so transfers are batched: weights are
shipped as eight j-strip (o-strip) tensors host-packed to [128, 8, 128]
so each strip is one DMA and gates exactly one accumulation group, x
arrives as one DMA per token tile (host-packed [128, 8, C]), and the
biases/gates land in one merged DMA each. Token tiles are 512 wide
(one fp32 PSUM bank) with a final 256-wide tile to trim padding (f32r
keeps full rate at free dim >= 256). A few dummy matmuls run in the
shadow of the initial DMA ramp to engage the PE clock-gate (HAM)
before the real matmuls arrive.
"""

import numpy as np

NUM_EXPERTS = 8
TOP_K = 2
D = 1024

_prog_cache = {}


def _plan_tiles(max_load):
    """Token-tile sizes covering max_load: 512s with a final 256 when it fits."""
    n256 = -(-max_load // 256)
    tiles = [512] * (n256 // 2)
    if n256 % 2 == 1:
        tiles.append(256)
    if not tiles:
        tiles = [256]
    return sum(tiles), tiles


def _build_program(tile_plan):
    """Build the per-core Bass program: one expert's MLP over C tokens."""
    from contextlib import ExitStack

    import concourse.tile as tile
    from concourse import bacc, mybir

    f32 = mybir.dt.float32
    f32r = mybir.dt.float32r
    ADD = mybir.AluOpType.add
    MAX = mybir.AluOpType.max
    MULT = mybir.AluOpType.mult

    C, tok_tiles = tile_plan

    nc = bacc.Bacc("TRN2", target_bir_lowering=False, debug=False,
                   num_devices=NUM_EXPERTS)

    # host-packed layouts (see _make_in_maps):
    #   xT:  [128, 8, C]      xT[p, d, c] = x_gathered[c, d*128+p]
    #   w1:  [8, 128, 8, 128] w1[j, p, d, r] = W1[d*128+p, j*128+r]
    #   w2:  [8, 128, 8, 128] w2[o, p, j, r] = W2[j*128+p, o*128+r]
    #   b1:  [128, 8]         b1[p, j] = b1[j*128+p]   (b2 same)
    #   yT:  [128, 8, C]      yT[p, o, c] = y[c, o*128+p] * gate[c]
    xT_d = nc.dram_tensor("xT", [128, 8, C], f32r, kind="ExternalInput").ap()
    w1_d = nc.dram_tensor("w1", [8, 128, 8, 128], f32r, kind="ExternalInput").ap()
    w2_d = nc.dram_tensor("w2", [8, 128, 8, 128], f32r, kind="ExternalInput").ap()
    bb_d = nc.dram_tensor("bb", [128, 16], f32, kind="ExternalInput").ap()
    go_d = nc.dram_tensor("go", [1, C + 128], f32r, kind="ExternalInput").ap()
    yT_d = nc.dram_tensor("yT", [128, 8, C], f32, kind="ExternalOutput").ap()

    with tile.TileContext(nc) as tc, ExitStack() as ctx:
        wpool = ctx.enter_context(tc.tile_pool(name="w", bufs=1))
        cpool = ctx.enter_context(tc.tile_pool(name="const", bufs=1))
        xpool = ctx.enter_context(tc.tile_pool(name="x", bufs=2))
        hpool = ctx.enter_context(tc.tile_pool(name="h", bufs=2))
        ypool = ctx.enter_context(tc.tile_pool(name="y", bufs=2))
        gpool = ctx.enter_context(tc.tile_pool(name="g", bufs=2))
        php = ctx.enter_context(tc.tile_pool(name="ph", bufs=3, space="PSUM"))
        pyp = ctx.enter_context(tc.tile_pool(name="py", bufs=3, space="PSUM"))
        pgp = ctx.enter_context(tc.tile_pool(name="pg", bufs=2, space="PSUM"))

        # tiny constants on the ACT DGE queue (parallel with the weight
        # stream on the SP queue), merged into single transfers:
        # bb = [b1 | b2] per-partition, go = [gate row | ones row]
        bb_sb = cpool.tile([128, 16], f32, tag="bb")
        nc.sync.dma_start(bb_sb[:], bb_d[:])
        b1_sb = bb_sb[:, 0:8]
        b2_sb = bb_sb[:, 8:16]
        go_sb = cpool.tile([1, C + 128], f32r, tag="go")
        nc.sync.dma_start(go_sb[:], go_d[:])
        g_sb = go_sb[:, 0:C]
        ones_sb = go_sb[:, C:C + 128]

        # PE warm-up in the shadow of the initial DMA ramp: ~4us of dummy
        # K=1 matmuls (gated only on the tiny g/ones transfers) keep the
        # HAM activity monitor busy so the real matmuls run at 2.4 GHz
        warm = pgp.tile([128, min(C, 512)], f32, tag="gps")
        for _ in range(6):
            nc.tensor.matmul(warm[:], ones_sb[:], g_sb[:, 0:min(C, 512)],
                             start=True, stop=True)

        # DMA emission in consumption order: w1 strip 0, then the first
        # token tile of x^T per d-block (the j=0 group's d-MMs start as each
        # block lands), then the remaining w1 strips (one gates each j-group)
        TT0 = tok_tiles[0]
        w1_sb = [None] * 8
        w1_first = wpool.tile([128, 8 * 128], f32r, tag="w1_0")
        nc.sync.dma_start(w1_first[:], w1_d[0])
        w1_sb[0] = w1_first
        x0a = xpool.tile([128, 4 * TT0], f32r, tag="x0a")
        nc.sync.dma_start(x0a[:], xT_d[:, 0:4, 0:TT0])
        x0b = xpool.tile([128, 4 * TT0], f32r, tag="x0b")
        nc.sync.dma_start(x0b[:], xT_d[:, 4:8, 0:TT0])
        x_sb0 = (x0a, x0b)
        for j in range(1, 8):
            w1_strip = wpool.tile([128, 8 * 128], f32r, tag=f"w1_{j}")
            nc.sync.dma_start(w1_strip[:], w1_d[j])
            w1_sb[j] = w1_strip

        # w2 o-strips next: strip o gates tile 0's layer-2 o-group, which
        # starts right after layer 1 (~the w1 stream), so these must not
        # queue behind the second x tile; the second x tile (needed only
        # when tile 0 fully finishes) slots in before the last strip
        x_tiles = [None] * len(tok_tiles)
        x_tiles[0] = x_sb0
        w2_sb = [None] * 8
        for o in range(8):
            if o == 6 and len(tok_tiles) > 1:
                TT1 = tok_tiles[1]
                x1a = xpool.tile([128, 4 * TT1], f32r, tag="x0a")
                nc.sync.dma_start(x1a[:], xT_d[:, 0:4, TT0:TT0 + TT1])
                x_tiles[1] = (x1a, None)
            if o == 7 and len(tok_tiles) > 1:
                TT1 = tok_tiles[1]
                x1b = xpool.tile([128, 4 * TT1], f32r, tag="x0b")
                nc.sync.dma_start(x1b[:], xT_d[:, 4:8, TT0:TT0 + TT1])
                x_tiles[1] = (x_tiles[1][0], x1b)
            w2_strip = wpool.tile([128, 8 * 128], f32r, tag=f"w2_{o}")
            nc.sync.dma_start(w2_strip[:], w2_d[o])
            w2_sb[o] = w2_strip

        tile_pos = np.cumsum([0] + tok_tiles).tolist()
        pos = 0
        for t, TT in enumerate(tok_tiles):
            tsl = slice(pos, pos + TT)

            # prefetch x for tile t+1 (tiles 0 and 1 already issued)
            nt = t + 1
            if nt < len(tok_tiles) and x_tiles[nt] is None:
                x_pref = xpool.tile([128, 8 * tok_tiles[nt]], f32r, tag="x")
                nc.sync.dma_start(
                    x_pref[:],
                    xT_d[:, :, tile_pos[nt]:tile_pos[nt] + tok_tiles[nt]])
                x_tiles[nt] = x_pref

            x_sb = x_tiles[t]

            def xs(d):
                if isinstance(x_sb, tuple):
                    half = x_sb[d // 4]
                    return half[:, (d % 4) * TT:(d % 4 + 1) * TT]
                return x_sb[:, d * TT:(d + 1) * TT]

            # broadcast gate row across partitions: G[p, n] = g[n]
            g_ps = pgp.tile([128, TT], f32, tag="gps")
            nc.tensor.matmul(g_ps[:], ones_sb[:], g_sb[:, tsl],
                             start=True, stop=True)
            g_bc = gpool.tile([128, TT], f32, tag="gbc")
            nc.vector.tensor_copy(g_bc[:], g_ps[:])

            # layer 1: h^T[j,:] = relu(sum_d W1[d,j]^T x^T[d,:] + b1[j])
            h_sb = []
            for j in range(8):
                ph = php.tile([128, TT], f32, tag="ph")
                for d in range(8):
                    nc.tensor.matmul(ph[:],
                                     w1_sb[j][:, d * 128:(d + 1) * 128],
                                     xs(d),
                                     start=(d == 0), stop=(d == 7))
                ht = hpool.tile([128, TT], f32r, tag=f"h{j}")
                nc.vector.tensor_scalar(ht[:], ph[:], b1_sb[:, j:j + 1], 0.0,
                                        op0=ADD, op1=MAX)
                h_sb.append(ht)

            # layer 2 + gate: y^T[o,:] = (sum_j W2[j,o]^T h^T[j,:] + b2[o]) * g
            for o in range(8):
                py = pyp.tile([128, TT], f32, tag="py")
                for j in range(8):
                    nc.tensor.matmul(py[:],
                                     w2_sb[o][:, j * 128:(j + 1) * 128],
                                     h_sb[j][:],
                                     start=(j == 0), stop=(j == 7))
                yt = ypool.tile([128, TT], f32, tag=f"y{o}")
                nc.vector.scalar_tensor_tensor(yt[:], py[:], b2_sb[:, o:o + 1],
                                               g_bc[:], op0=ADD, op1=MULT)
                nc.sync.dma_start(yT_d[:, o, tsl], yt[:])

            pos += TT

    nc.compile()
    return nc


def _route(x, Wg, bg):
    """Host gating: fp32 softmax + top-2, matching jax.lax.top_k semantics."""
    logits = x @ Wg + bg
    m = logits.max(axis=1, keepdims=True)
    e = np.exp(logits - m)
    gates = e / e.sum(axis=1, keepdims=True)
    # stable argsort on negated values = ties broken by lower index (jax)
    order = np.argsort(-gates, axis=1, kind="stable")[:, :TOP_K]
    return gates, order


def _pack_w(W):
    """[1024, 1024] -> [8, 128, 8, 128]: strip s, part p, rowtile d, col r."""
    # out[s, p, d, r] = W[d*128+p, s*128+r]
    return np.ascontiguousarray(
        W.reshape(8, 128, 8, 128).transpose(2, 1, 0, 3))


def _make_in_maps(x, W1, b1, W2, b2, gates, order, tok_lists, C):
    in_maps = []
    for e in range(NUM_EXPERTS):
        toks = tok_lists[e]
        ne = len(toks)
        xT_e = np.zeros((128, 8, C), dtype=np.float32)
        # xT_e[p, d, :ne] = x[toks, d*128+p].T
        xT_e[:, :, :ne] = x[toks].T.reshape(8, 128, ne).transpose(1, 0, 2)
        g_e = np.zeros((1, C), dtype=np.float32)
        g_e[0, :ne] = gates[toks, e]
        in_maps.append({
            "xT": xT_e,
            "w1": _pack_w(W1[e]),
            "w2": _pack_w(W2[e]),
            "bb": np.ascontiguousarray(np.concatenate(
                [b1[e].reshape(8, 128).T, b2[e].reshape(8, 128).T], axis=1)),
            "go": np.concatenate(
                [g_e, np.ones((1, 128), dtype=np.float32)], axis=1),
        })
    return in_maps


def kernel(x, W1, b1, W2, b2, Wg, bg):
    from concourse import bass_utils

    x = np.ascontiguousarray(np.asarray(x, dtype=np.float32))
    W1 = np.asarray(W1, dtype=np.float32)
    b1 = np.asarray(b1, dtype=np.float32)
    W2 = np.asarray(W2, dtype=np.float32)
    b2 = np.asarray(b2, dtype=np.float32)
    Wg = np.asarray(Wg, dtype=np.float32)
    bg = np.asarray(bg, dtype=np.float32)
    n = x.shape[0]

    gates, order = _route(x, Wg, bg)
    tok_lists = [np.where((order == e).any(axis=1))[0] for e in range(NUM_EXPERTS)]
    max_load = max(len(t) for t in tok_lists)
    C, tok_tiles = _plan_tiles(max_load)

    key = (C, tuple(tok_tiles))
    if key not in _prog_cache:
        _prog_cache[key] = _build_program((C, tok_tiles))
    nc = _prog_cache[key]

    in_maps = _make_in_maps(x, W1, b1, W2, b2, gates, order, tok_lists, C)
    res = bass_utils.run_bass_kernel_spmd(nc, in_maps, list(range(NUM_EXPERTS)))
    # yT result: [128, 8, C] -> y_e[c, o*128+p] = yT[p, o, c]
    yT_all = np.stack([res.results[e]["yT"] for e in range(NUM_EXPERTS)])

    # scatter-add the two expert contributions per token (already gated)
    slot = np.zeros((NUM_EXPERTS, n), dtype=np.int64)
    for e in range(NUM_EXPERTS):
        slot[e, tok_lists[e]] = np.arange(len(tok_lists[e]))
    rows = np.arange(n)
    # gather columns: result [n, 128, 8] -> reshape to [n, 1024]
    out = np.zeros((n, D), dtype=np.float32)
    for k in range(TOP_K):
        ek = order[:, k]
        picked = yT_all[ek, :, :, slot[ek, rows]]   # [n, 128, 8]
        out += picked.transpose(0, 2, 1).reshape(n, D)
    return out



# revision 5
# speedup vs baseline: 1.3023x; 1.3023x over previous
"""Trainium2 Bass kernel for an 8-expert top-2 MoE layer.

Strategy (expert-parallel, per the sharding hint): the host computes the
tiny gating matmul + softmax + top-2 routing, gathers each expert's
assigned tokens, and ships one expert per NeuronCore. Each core runs the
heavy 2-layer MLP for its expert over its assigned tokens, applies the
gate weights on-device, and the host scatter-adds the two expert
contributions per token.

The MLP matmuls run as fp8(e4m3) DoubleRow pair-matmuls (each
instruction contracts K=256 = 2 k-tiles at half-rate-per-row), with
*residual compensation* to keep accuracy: every operand A is shipped as
a hi/lo pair (A_hi = fp8(A), A_lo = fp8(A - A_hi), same scale), and each
1024-contraction runs three streams

    A_hi @ W_hi  +  A_hi @ W_lo  +  A_lo @ W_hi

which costs 12 pair-matmuls per 128-wide output group (vs 8 full-rate
matmuls for f32r) -> 0.75 cycles/row/layer equivalent, and leaves only
residual-of-residual error (~3e-3 max-rel, gate is 2e-2).

Scales are powers of two folded into host-prepped constants:
  W1 is shipped as fp8(64*W1), so PSUM1 = 64*(x@W1);
  h is evicted as relu(PSUM1 + 64*b1) = 64*h (max ~206 < 240 = e4m3 max)
  via one ACT relu (bias AP), then cast to fp8 (hi) on ACT and the
  residual (lo) computed on DVE;
  W2 is shipped as fp8(128*W2), so PSUM2 = 8192*(h@W2), and the y
  eviction folds b2*8192 and gate/8192 into the baseline's single
  (psum + b2') * gate' DVE op; y ships bf16.

The token dimension stays on the moving/free side: tiles of 512 tokens
(one fp32 PSUM bank) plus an exact-size tail tile (fp8 has no >=256
free-dim requirement), so C = 4*512 + (max_load - 2048) instead of
rounding up to 256. Weights land as one 2KB-per-partition DMA strip per
output group (hi and lo packed together); x arrives as one DMA per
(tile, hi/lo); y leaves as one DMA per tile. A few dummy f32r matmuls
run in the shadow of the initial DMA ramp to engage the PE clock-gate
before the real matmuls arrive.
"""

import numpy as np

NUM_EXPERTS = 8
TOP_K = 2
D = 1024
S1 = 64.0     # W1/h scale
S2 = 128.0    # W2 scale (gate folds 1/(S1*S2))

_prog_cache = {}


def _plan_tiles(max_load):
    """Token-tile sizes covering max_load: 512s plus a tail (multiple of 8
    to satisfy fp32r ISA restrictions on the gate-broadcast matmul)."""
    n512, rem = divmod(max(max_load, 1), 512)
    tiles = [512] * n512
    if rem:
        tiles.append(-(-rem // 8) * 8)
    return sum(tiles), tiles


def _build_program(tile_plan):
    """Build the per-core Bass program: one expert's MLP over C tokens."""
    from contextlib import ExitStack

    import concourse.tile as tile
    from concourse import bacc, mybir

    f32 = mybir.dt.float32
    f32r = mybir.dt.float32r
    f8 = mybir.dt.float8e4
    bf16 = mybir.dt.bfloat16
    DR = mybir.MatmulPerfMode.DoubleRow
    ADD = mybir.AluOpType.add
    MAX = mybir.AluOpType.max
    MULT = mybir.AluOpType.mult
    RELU = mybir.ActivationFunctionType.Relu
    COPY = mybir.ActivationFunctionType.Copy

    C, tok_tiles = tile_plan

    nc = bacc.Bacc("TRN2", target_bir_lowering=False, debug=False,
                   num_devices=NUM_EXPERTS)

    # host-packed layouts (see _make_in_maps), all e4m3 except consts:
    #   xh/xl: [128, 8, C]          x[p, d, c] = q(x_gathered[c, d*128+p]) hi/lo
    #   w1:  [8, 128, 2, 8, 128]    w1[j, p, v, d, r] = q(64*W1[d*128+p, j*128+r]) hi/lo
    #   w2:  [8, 128, 2, 8, 128]    w2[o, p, v, j, r] = q(128*W2[j*128+p, o*128+r])
    #   bb:  [128, 16] f32          [64*b1 | 8192*b2] per-partition
    #   go:  [1, C+128] f32r        [gate row / 8192 | ones row]
    #   yT:  [128, 8, C] bf16       yT[p, o, c] = y[c, o*128+p] * gate[c]
    xh_d = nc.dram_tensor("xh", [128, 8, C], f8, kind="ExternalInput").ap()
    xl_d = nc.dram_tensor("xl", [128, 8, C], f8, kind="ExternalInput").ap()
    w1_d = nc.dram_tensor("w1", [8, 128, 2, 8, 128], f8, kind="ExternalInput").ap()
    w2_d = nc.dram_tensor("w2", [8, 128, 2, 8, 128], f8, kind="ExternalInput").ap()
    bb_d = nc.dram_tensor("bb", [128, 16], f32, kind="ExternalInput").ap()
    go_d = nc.dram_tensor("go", [1, C + 128], f32r, kind="ExternalInput").ap()
    yT_d = nc.dram_tensor("yT", [128, 8, C], bf16, kind="ExternalOutput").ap()

    with tile.TileContext(nc) as tc, ExitStack() as ctx:
        wpool = ctx.enter_context(tc.tile_pool(name="w", bufs=1))
        cpool = ctx.enter_context(tc.tile_pool(name="const", bufs=1))
        xpool = ctx.enter_context(tc.tile_pool(name="x", bufs=2))
        hxpool = ctx.enter_context(tc.tile_pool(name="hx", bufs=3))
        hpool = ctx.enter_context(tc.tile_pool(name="h", bufs=2))
        ypool = ctx.enter_context(tc.tile_pool(name="y", bufs=2))
        gpool = ctx.enter_context(tc.tile_pool(name="g", bufs=2))
        php = ctx.enter_context(tc.tile_pool(name="ph", bufs=3, space="PSUM"))
        pyp = ctx.enter_context(tc.tile_pool(name="py", bufs=3, space="PSUM"))
        pgp = ctx.enter_context(tc.tile_pool(name="pg", bufs=2, space="PSUM"))

        # tiny constants first (cheap DMAs that gate the warm-up matmuls)
        bb_sb = cpool.tile([128, 16], f32, tag="bb")
        nc.sync.dma_start(bb_sb[:], bb_d[:])
        go_sb = cpool.tile([1, C + 128], f32r, tag="go")
        nc.sync.dma_start(go_sb[:], go_d[:])
        g_sb = go_sb[:, 0:C]
        ones_sb = go_sb[:, C:C + 128]

        # PE warm-up in the shadow of the initial DMA ramp: dummy K=1
        # matmuls keep the PE busy so the cost-model p-state (and the HW
        # HAM clock gate) reaches full speed before the real matmuls
        warm = pgp.tile([128, min(C, 512)], f32, tag="gps")
        for _ in range(6):
            nc.tensor.matmul(warm[:], ones_sb[:], g_sb[:, 0:min(C, 512)],
                             start=True, stop=True)

        # DMA emission in consumption order: w1 strip 0, first token tile
        # of xh/xl, remaining w1 strips, then w2 strips (tile-1 x slots in
        # before the last strips so it can't starve tile-0's layer 2)
        TT0 = tok_tiles[0]
        w1_sb = [None] * 8
        w1_first = wpool.tile([128, 2, 8, 128], f8, tag="w1_0")
        nc.sync.dma_start(w1_first[:], w1_d[0])
        w1_sb[0] = w1_first
        xh0 = xpool.tile([128, 8, TT0], f8, tag="xh")
        nc.sync.dma_start(xh0[:], xh_d[:, :, 0:TT0])
        xl0 = xpool.tile([128, 8, TT0], f8, tag="xl")
        nc.sync.dma_start(xl0[:], xl_d[:, :, 0:TT0])
        for j in range(1, 8):
            w1_strip = wpool.tile([128, 2, 8, 128], f8, tag=f"w1_{j}")
            nc.sync.dma_start(w1_strip[:], w1_d[j])
            w1_sb[j] = w1_strip

        x_tiles = [None] * len(tok_tiles)
        x_tiles[0] = (xh0, xl0)
        w2_sb = [None] * 8
        for o in range(8):
            if o == 6 and len(tok_tiles) > 1:
                TT1 = tok_tiles[1]
                xh1 = xpool.tile([128, 8, TT1], f8, tag="xh")
                nc.sync.dma_start(xh1[:], xh_d[:, :, TT0:TT0 + TT1])
                x_tiles[1] = (xh1, None)
            if o == 7 and len(tok_tiles) > 1:
                TT1 = tok_tiles[1]
                xl1 = xpool.tile([128, 8, TT1], f8, tag="xl")
                nc.sync.dma_start(xl1[:], xl_d[:, :, TT0:TT0 + TT1])
                x_tiles[1] = (x_tiles[1][0], xl1)
            w2_strip = wpool.tile([128, 2, 8, 128], f8, tag=f"w2_{o}")
            nc.sync.dma_start(w2_strip[:], w2_d[o])
            w2_sb[o] = w2_strip

        tile_pos = np.cumsum([0] + tok_tiles).tolist()
        pos = 0
        for t, TT in enumerate(tok_tiles):
            tsl = slice(pos, pos + TT)

            # prefetch x for tile t+1 (tiles 0 and 1 already issued)
            nt = t + 1
            if nt < len(tok_tiles) and x_tiles[nt] is None:
                TTn = tok_tiles[nt]
                nsl = slice(tile_pos[nt], tile_pos[nt] + TTn)
                xh_p = xpool.tile([128, 8, TTn], f8, tag="xh")
                nc.sync.dma_start(xh_p[:], xh_d[:, :, nsl])
                xl_p = xpool.tile([128, 8, TTn], f8, tag="xl")
                nc.sync.dma_start(xl_p[:], xl_d[:, :, nsl])
                x_tiles[nt] = (xh_p, xl_p)

            xh_sb, xl_sb = x_tiles[t]

            # broadcast gate row across partitions: G[p, n] = g[n]
            g_ps = pgp.tile([128, TT], f32, tag="gps")
            nc.tensor.matmul(g_ps[:], ones_sb[:], g_sb[:, tsl],
                             start=True, stop=True)
            g_bc = gpool.tile([128, TT], f32, tag="gbc")
            nc.vector.tensor_copy(g_bc[:], g_ps[:])

            # layer 1: 64*h^T[j] = relu(64*sum_d W1[d,j]^T x^T[d] + 64*b1[j])
            # 3 fp8 DoubleRow streams: xh@W1h + xh@W1l + xl@W1h
            h8 = [hpool.tile([128, 2, TT], f8, tag=f"h8_{q}", name=f"h8_{q}")
                  for q in range(4)]
            hl8 = [hpool.tile([128, 2, TT], f8, tag=f"hl8_{q}", name=f"hl8_{q}")
                   for q in range(4)]
            for j in range(8):
                ph = php.tile([128, TT], f32, tag="ph")
                n = 0
                for v, xs in ((0, xh_sb), (1, xh_sb), (0, xl_sb)):
                    for q in range(4):
                        nc.tensor.matmul(ph[:],
                                         w1_sb[j][:, v, 2 * q:2 * q + 2, :],
                                         xs[:, 2 * q:2 * q + 2, :],
                                         start=(n == 0), stop=(n == 11),
                                         perf_mode=DR)
                        n += 1
                hx32 = hxpool.tile([128, TT], f32, tag="hx32")
                nc.scalar.activation(hx32[:], ph[:], RELU,
                                     bias=bb_sb[:, j:j + 1], scale=1.0)
                h8s = h8[j // 2][:, j % 2, :]
                nc.scalar.activation(h8s, hx32[:], COPY)
                nc.vector.scalar_tensor_tensor(hl8[j // 2][:, j % 2, :],
                                               h8s, -1.0, hx32[:],
                                               op0=MULT, op1=ADD)

            # layer 2 + gate: y^T[o] = (sum_j W2[j,o]^T h^T[j] + b2[o]) * g
            # 3 fp8 DoubleRow streams: h8@W2h + h8@W2l + hl8@W2h
            ybig = ypool.tile([128, 8, TT], bf16, tag="y")
            for o in range(8):
                py = pyp.tile([128, TT], f32, tag="py")
                n = 0
                for v, hs in ((0, h8), (1, h8), (0, hl8)):
                    for q in range(4):
                        nc.tensor.matmul(py[:],
                                         w2_sb[o][:, v, 2 * q:2 * q + 2, :],
                                         hs[q][:],
                                         start=(n == 0), stop=(n == 11),
                                         perf_mode=DR)
                        n += 1
                nc.vector.scalar_tensor_tensor(ybig[:, o, :], py[:],
                                               bb_sb[:, 8 + o:9 + o],
                                               g_bc[:], op0=ADD, op1=MULT)
            nc.sync.dma_start(yT_d[:, :, tsl], ybig[:])

            pos += TT

    nc.compile()
    return nc


def _route(x, Wg, bg):
    """Host gating: fp32 softmax + top-2, matching jax.lax.top_k semantics."""
    logits = x @ Wg + bg
    m = logits.max(axis=1, keepdims=True)
    e = np.exp(logits - m)
    gates = e / e.sum(axis=1, keepdims=True)
    # stable argsort on negated values = ties broken by lower index (jax)
    order = np.argsort(-gates, axis=1, kind="stable")[:, :TOP_K]
    return gates, order


def _q8(a):
    import ml_dtypes
    return np.asarray(a).astype(ml_dtypes.float8_e4m3)


def _pack_w(W, scale):
    """[1024,1024] -> [128, 2, 8, 128] strips stacked [8,...]: hi/lo fp8.

    out[s, p, v, d, r] = q_v(scale * W[d*128+p, s*128+r])
    """
    Ws = (W * scale).astype(np.float32)
    Wh = _q8(Ws)
    Wl = _q8(Ws - Wh.astype(np.float32))
    packs = []
    for Wv in (Wh, Wl):
        # [d, p, s, r] -> [s, p, d, r]
        packs.append(Wv.reshape(8, 128, 8, 128).transpose(2, 1, 0, 3))
    # -> [s, p, v, d, r]
    return np.ascontiguousarray(np.stack(packs, axis=2))


def _make_in_maps(x, W1, b1, W2, b2, gates, order, tok_lists, C):
    xh_full = _q8(x)
    xl_full = _q8(x - xh_full.astype(np.float32))
    in_maps = []
    for e in range(NUM_EXPERTS):
        toks = tok_lists[e]
        ne = len(toks)
        xs = {}
        for name, xq in (("xh", xh_full), ("xl", xl_full)):
            xT_e = np.zeros((128, 8, C), dtype=xq.dtype)
            # xT_e[p, d, :ne] = xq[toks, d*128+p].T
            xT_e[:, :, :ne] = xq[toks].T.reshape(8, 128, ne).transpose(1, 0, 2)
            xs[name] = xT_e
        g_e = np.zeros((1, C), dtype=np.float32)
        g_e[0, :ne] = gates[toks, e] / (S1 * S2)
        in_maps.append({
            **xs,
            "w1": _pack_w(W1[e], S1),
            "w2": _pack_w(W2[e], S2),
            "bb": np.ascontiguousarray(np.concatenate(
                [(S1 * b1[e]).reshape(8, 128).T,
                 (S1 * S2 * b2[e]).reshape(8, 128).T], axis=1)),
            "go": np.concatenate(
                [g_e, np.ones((1, 128), dtype=np.float32)], axis=1),
        })
    return in_maps


def kernel(x, W1, b1, W2, b2, Wg, bg):
    from concourse import bass_utils

    x = np.ascontiguousarray(np.asarray(x, dtype=np.float32))
    W1 = np.asarray(W1, dtype=np.float32)
    b1 = np.asarray(b1, dtype=np.float32)
    W2 = np.asarray(W2, dtype=np.float32)
    b2 = np.asarray(b2, dtype=np.float32)
    Wg = np.asarray(Wg, dtype=np.float32)
    bg = np.asarray(bg, dtype=np.float32)
    n = x.shape[0]

    gates, order = _route(x, Wg, bg)
    tok_lists = [np.where((order == e).any(axis=1))[0] for e in range(NUM_EXPERTS)]
    max_load = max(len(t) for t in tok_lists)
    C, tok_tiles = _plan_tiles(max_load)

    key = (C, tuple(tok_tiles))
    if key not in _prog_cache:
        _prog_cache[key] = _build_program((C, tok_tiles))
    nc = _prog_cache[key]

    in_maps = _make_in_maps(x, W1, b1, W2, b2, gates, order, tok_lists, C)
    res = bass_utils.run_bass_kernel_spmd(nc, in_maps, list(range(NUM_EXPERTS)))
    # yT result: [128, 8, C] bf16 -> y_e[c, o*128+p] = yT[p, o, c]
    yT_all = np.stack([res.results[e]["yT"].astype(np.float32)
                       for e in range(NUM_EXPERTS)])

    # scatter-add the two expert contributions per token (already gated)
    slot = np.zeros((NUM_EXPERTS, n), dtype=np.int64)
    for e in range(NUM_EXPERTS):
        slot[e, tok_lists[e]] = np.arange(len(tok_lists[e]))
    rows = np.arange(n)
    out = np.zeros((n, D), dtype=np.float32)
    for k in range(TOP_K):
        ek = order[:, k]
        picked = yT_all[ek, :, :, slot[ek, rows]]   # [n, 128, 8]
        out += picked.transpose(0, 2, 1).reshape(n, D)
    return out


# revision 8
# speedup vs baseline: 1.3739x; 1.0550x over previous
"""Trainium2 Bass kernel for an 8-expert top-2 MoE layer.

Strategy (expert-parallel, per the sharding hint): the host computes the
tiny gating matmul + softmax + top-2 routing, gathers each expert's
assigned tokens, and ships one expert per NeuronCore. Each core runs the
heavy 2-layer MLP for its expert over its assigned tokens, applies the
gate weights on-device, and the host scatter-adds the two expert
contributions per token.

The MLP matmuls run as fp8(e4m3) DoubleRow pair-matmuls (each
instruction contracts K=256 = 2 k-tiles at half-rate-per-row), with
*residual compensation* to keep accuracy: every operand A is shipped as
a hi/lo pair (A_hi = fp8(A), A_lo = fp8(A - A_hi), same scale), and each
1024-contraction runs three streams

    A_hi @ W_hi  +  A_hi @ W_lo  +  A_lo @ W_hi

which costs 12 pair-matmuls per 128-wide output group (vs 8 full-rate
matmuls for f32r) -> 0.75 cycles/row/layer equivalent, and leaves only
residual-of-residual error (~3e-3 max-rel, gate is 2e-2).

Scales are powers of two folded into host-prepped constants:
  W1 is shipped as fp8(64*W1), so PSUM1 = 64*(x@W1);
  h is evicted as relu(PSUM1 + 64*b1) = 64*h (max ~206 < 240 = e4m3 max)
  via one ACT relu (bias AP), then cast to fp8 (hi) on ACT and the
  residual (lo) computed on DVE;
  W2 is shipped as fp8(128*W2), so PSUM2 = 8192*(h@W2), and the y
  eviction folds b2*8192 and gate/8192 into the baseline's single
  (psum + b2') * gate' DVE op; y ships bf16.

The token dimension stays on the moving/free side: tiles of 512 tokens
(one fp32 PSUM bank) plus an exact-size tail tile (fp8 has no >=256
free-dim requirement), so C = 4*512 + (max_load - 2048) instead of
rounding up to 256. Weights land as one 2KB-per-partition DMA strip per
output group (hi and lo packed together); x arrives as one DMA per
(tile, hi/lo); y leaves as one DMA per tile. A few dummy f32r matmuls
run in the shadow of the initial DMA ramp to engage the PE clock-gate
before the real matmuls arrive.
"""

import numpy as np

NUM_EXPERTS = 8
TOP_K = 2
D = 1024
S1 = 64.0     # W1/h scale
S2 = 128.0    # W2 scale (gate folds 1/(S1*S2))

_prog_cache = {}


def _plan_tiles(max_load):
    """Token-tile sizes covering max_load: a small first tile (so the first
    x DMA lands early), then 512s, then an exact-ish tail (multiple of 8 to
    satisfy fp32r ISA restrictions on the gate-broadcast matmul)."""
    r8 = lambda v: -(-v // 8) * 8
    if max_load <= 256:
        tiles = [r8(max(max_load, 8))]
    else:
        n512, rem = divmod(max_load - 256, 512)
        tiles = [256] + [512] * n512
        if rem:
            tiles.append(r8(rem))
    return sum(tiles), tiles


def _build_program(tile_plan):
    """Build the per-core Bass program: one expert's MLP over C tokens."""
    from contextlib import ExitStack

    import concourse.tile as tile
    from concourse import bacc, mybir

    f32 = mybir.dt.float32
    f32r = mybir.dt.float32r
    f8 = mybir.dt.float8e4
    bf16 = mybir.dt.bfloat16
    DR = mybir.MatmulPerfMode.DoubleRow
    ADD = mybir.AluOpType.add
    MAX = mybir.AluOpType.max
    MULT = mybir.AluOpType.mult
    RELU = mybir.ActivationFunctionType.Relu
    COPY = mybir.ActivationFunctionType.Copy

    C, tok_tiles = tile_plan

    nc = bacc.Bacc("TRN2", target_bir_lowering=False, debug=False,
                   num_devices=NUM_EXPERTS)

    # host-packed layouts (see _make_in_maps), all e4m3 except consts:
    #   xh/xl: [128, 8, C]          x[p, d, c] = q(x_gathered[c, d*128+p]) hi/lo
    #   w1:  [8, 128, 2, 8, 128]    w1[j, p, v, d, r] = q(64*W1[d*128+p, j*128+r]) hi/lo
    #   w2:  [8, 128, 2, 8, 128]    w2[o, p, v, j, r] = q(128*W2[j*128+p, o*128+r])
    #   bb:  [128, 16] f32          [64*b1 | 8192*b2] per-partition
    #   go:  [1, C+128] f32r        [gate row / 8192 | ones row]
    #   yT:  [128, 8, C] bf16       yT[p, o, c] = y[c, o*128+p] * gate[c]
    xh_d = nc.dram_tensor("xh", [128, 8, C], f8, kind="ExternalInput").ap()
    xl_d = nc.dram_tensor("xl", [128, 8, C], f8, kind="ExternalInput").ap()
    w1_d = nc.dram_tensor("w1", [8, 128, 2, 8, 128], f8, kind="ExternalInput").ap()
    w2_d = nc.dram_tensor("w2", [8, 128, 2, 8, 128], f8, kind="ExternalInput").ap()
    bb_d = nc.dram_tensor("bb", [128, 16], f32, kind="ExternalInput").ap()
    go_d = nc.dram_tensor("go", [1, C + 128], f32r, kind="ExternalInput").ap()
    yT_d = nc.dram_tensor("yT", [128, 8, C], bf16, kind="ExternalOutput").ap()

    with tile.TileContext(nc) as tc, ExitStack() as ctx:
        wpool = ctx.enter_context(tc.tile_pool(name="w", bufs=1))
        cpool = ctx.enter_context(tc.tile_pool(name="const", bufs=1))
        xpool = ctx.enter_context(tc.tile_pool(name="x", bufs=2))
        hxpool = ctx.enter_context(tc.tile_pool(name="hx", bufs=3))
        hpool = ctx.enter_context(tc.tile_pool(name="h", bufs=2))
        ypool = ctx.enter_context(tc.tile_pool(name="y", bufs=2))
        gpool = ctx.enter_context(tc.tile_pool(name="g", bufs=2))
        php = ctx.enter_context(tc.tile_pool(name="ph", bufs=3, space="PSUM"))
        pyp = ctx.enter_context(tc.tile_pool(name="py", bufs=3, space="PSUM"))
        pgp = ctx.enter_context(tc.tile_pool(name="pg", bufs=2, space="PSUM"))

        # PE warm-up fed by a memset (no DMA dependency): dummy K=1 f32r
        # matmuls keep the PE busy from ~0 so the cost-model p-state (and
        # the HW HAM clock gate) reaches full speed before the real
        # matmuls, which queue behind them in PE order.
        wsrc = cpool.tile([1, 640], bf16, tag="wsrc")
        nc.vector.memset(wsrc[:], 1.0)
        warm = pgp.tile([128, 512], f32, tag="gps")
        for _ in range(8):
            nc.tensor.matmul(warm[:], wsrc[:, 0:128], wsrc[:, 128:640],
                             start=True, stop=True)

        # DMA emission in consumption order: w1 strip 0, first token tile
        # of xh/xl, constants, remaining w1 strips, then w2 strips (tile-1
        # x slots in before the last strips so it can't starve layer 2)
        TT0 = tok_tiles[0]
        w1_sb = [None] * 8
        w1_first = wpool.tile([128, 2, 8, 128], f8, tag="w1_0")
        nc.sync.dma_start(w1_first[:], w1_d[0])
        w1_sb[0] = w1_first
        xh0 = xpool.tile([128, 8, TT0], f8, tag="xh")
        nc.sync.dma_start(xh0[:], xh_d[:, :, 0:TT0])
        xl0 = xpool.tile([128, 8, TT0], f8, tag="xl")
        nc.sync.dma_start(xl0[:], xl_d[:, :, 0:TT0])
        bb_sb = cpool.tile([128, 16], f32, tag="bb")
        nc.sync.dma_start(bb_sb[:], bb_d[:])
        go_sb = cpool.tile([1, C + 128], f32r, tag="go")
        nc.sync.dma_start(go_sb[:], go_d[:])
        g_sb = go_sb[:, 0:C]
        ones_sb = go_sb[:, C:C + 128]
        for j in range(1, 8):
            w1_strip = wpool.tile([128, 2, 8, 128], f8, tag=f"w1_{j}")
            nc.sync.dma_start(w1_strip[:], w1_d[j])
            w1_sb[j] = w1_strip

        x_tiles = [None] * len(tok_tiles)
        x_tiles[0] = (xh0, xl0)
        w2_sb = [None] * 8
        for o in range(8):
            if o == 6 and len(tok_tiles) > 1:
                TT1 = tok_tiles[1]
                xh1 = xpool.tile([128, 8, TT1], f8, tag="xh")
                nc.sync.dma_start(xh1[:], xh_d[:, :, TT0:TT0 + TT1])
                x_tiles[1] = (xh1, None)
            if o == 7 and len(tok_tiles) > 1:
                TT1 = tok_tiles[1]
                xl1 = xpool.tile([128, 8, TT1], f8, tag="xl")
                nc.sync.dma_start(xl1[:], xl_d[:, :, TT0:TT0 + TT1])
                x_tiles[1] = (x_tiles[1][0], xl1)
            w2_strip = wpool.tile([128, 2, 8, 128], f8, tag=f"w2_{o}")
            nc.sync.dma_start(w2_strip[:], w2_d[o])
            w2_sb[o] = w2_strip

        tile_pos = np.cumsum([0] + tok_tiles).tolist()
        ntile = len(tok_tiles)
        h_tiles = [None] * ntile
        g_tiles = [None] * ntile

        def emit_l1(t):
            """Gate broadcast + layer 1 of tile t; leaves h8/hl8 + g_bc."""
            TT = tok_tiles[t]
            tsl = slice(tile_pos[t], tile_pos[t] + TT)

            # prefetch x for tile t+1 (tiles 0 and 1 issued upfront)
            nt = t + 1
            if nt < ntile and x_tiles[nt] is None:
                TTn = tok_tiles[nt]
                nsl = slice(tile_pos[nt], tile_pos[nt] + TTn)
                xh_p = xpool.tile([128, 8, TTn], f8, tag="xh")
                nc.sync.dma_start(xh_p[:], xh_d[:, :, nsl])
                xl_p = xpool.tile([128, 8, TTn], f8, tag="xl")
                nc.sync.dma_start(xl_p[:], xl_d[:, :, nsl])
                x_tiles[nt] = (xh_p, xl_p)

            xh_sb, xl_sb = x_tiles[t]

            # layer 1: 64*h^T[j] = relu(64*sum_d W1[d,j]^T x^T[d] + 64*b1[j])
            # 3 fp8 DoubleRow streams: xh@W1h + xh@W1l + xl@W1h
            h8 = [hpool.tile([128, 2, TT], f8, tag=f"h8_{q}", name=f"h8_{q}")
                  for q in range(4)]
            hl8 = [hpool.tile([128, 2, TT], f8, tag=f"hl8_{q}", name=f"hl8_{q}")
                   for q in range(4)]
            for j in range(8):
                ph = php.tile([128, TT], f32, tag="ph")
                n = 0
                for v, xs in ((0, xh_sb), (1, xh_sb), (0, xl_sb)):
                    for q in range(4):
                        nc.tensor.matmul(ph[:],
                                         w1_sb[j][:, v, 2 * q:2 * q + 2, :],
                                         xs[:, 2 * q:2 * q + 2, :],
                                         start=(n == 0), stop=(n == 11),
                                         perf_mode=DR)
                        n += 1
                hx32 = hxpool.tile([128, TT], f32, tag="hx32")
                nc.scalar.activation(hx32[:], ph[:], RELU,
                                     bias=bb_sb[:, j:j + 1], scale=1.0)
                h8s = h8[j // 2][:, j % 2, :]
                nc.scalar.activation(h8s, hx32[:], COPY)
                nc.vector.scalar_tensor_tensor(hl8[j // 2][:, j % 2, :],
                                               h8s, -1.0, hx32[:],
                                               op0=MULT, op1=ADD)
            h_tiles[t] = (h8, hl8)

            # broadcast gate row across partitions: G[p, n] = g[n]
            g_ps = pgp.tile([128, TT], f32, tag="gps")
            nc.tensor.matmul(g_ps[:], ones_sb[:], g_sb[:, tsl],
                             start=True, stop=True)
            g_bc = gpool.tile([128, TT], f32, tag="gbc")
            nc.vector.tensor_copy(g_bc[:], g_ps[:])
            g_tiles[t] = g_bc

        def emit_l2(t):
            """Layer 2 + gate + output DMA of tile t."""
            TT = tok_tiles[t]
            tsl = slice(tile_pos[t], tile_pos[t] + TT)
            h8, hl8 = h_tiles[t]
            g_bc = g_tiles[t]

            # layer 2 + gate: y^T[o] = (sum_j W2[j,o]^T h^T[j] + b2[o]) * g
            # 3 fp8 DoubleRow streams: h8@W2h + h8@W2l + hl8@W2h
            ybig = ypool.tile([128, 8, TT], bf16, tag="y")
            for o in range(8):
                py = pyp.tile([128, TT], f32, tag="py")
                n = 0
                for v, hs in ((0, h8), (1, h8), (0, hl8)):
                    for q in range(4):
                        nc.tensor.matmul(py[:],
                                         w2_sb[o][:, v, 2 * q:2 * q + 2, :],
                                         hs[q][:],
                                         start=(n == 0), stop=(n == 11),
                                         perf_mode=DR)
                        n += 1
                nc.vector.scalar_tensor_tensor(ybig[:, o, :], py[:],
                                               bb_sb[:, 8 + o:9 + o],
                                               g_bc[:], op0=ADD, op1=MULT)
            nc.sync.dma_start(yT_d[:, :, tsl], ybig[:])

        # software pipeline: layer 1 of tile t+1 runs (on PE) before layer
        # 2 of tile t, so the h-eviction chain (ACT relu -> ACT fp8 cast ->
        # DVE residual) of tile t hides under tile t+1's layer-1 matmuls.
        emit_l1(0)
        for t in range(1, ntile):
            emit_l1(t)
            emit_l2(t - 1)
        emit_l2(ntile - 1)

    nc.compile()
    return nc


def _route(x, Wg, bg):
    """Host gating: fp32 softmax + top-2, matching jax.lax.top_k semantics."""
    logits = x @ Wg + bg
    m = logits.max(axis=1, keepdims=True)
    e = np.exp(logits - m)
    gates = e / e.sum(axis=1, keepdims=True)
    # stable argsort on negated values = ties broken by lower index (jax)
    order = np.argsort(-gates, axis=1, kind="stable")[:, :TOP_K]
    return gates, order


def _q8(a):
    import ml_dtypes
    return np.asarray(a).astype(ml_dtypes.float8_e4m3)


def _pack_w(W, scale):
    """[1024,1024] -> [128, 2, 8, 128] strips stacked [8,...]: hi/lo fp8.

    out[s, p, v, d, r] = q_v(scale * W[d*128+p, s*128+r])
    """
    Ws = (W * scale).astype(np.float32)
    Wh = _q8(Ws)
    Wl = _q8(Ws - Wh.astype(np.float32))
    packs = []
    for Wv in (Wh, Wl):
        # [d, p, s, r] -> [s, p, d, r]
        packs.append(Wv.reshape(8, 128, 8, 128).transpose(2, 1, 0, 3))
    # -> [s, p, v, d, r]
    return np.ascontiguousarray(np.stack(packs, axis=2))


def _make_in_maps(x, W1, b1, W2, b2, gates, order, tok_lists, C):
    xh_full = _q8(x)
    xl_full = _q8(x - xh_full.astype(np.float32))
    in_maps = []
    for e in range(NUM_EXPERTS):
        toks = tok_lists[e]
        ne = len(toks)
        xs = {}
        for name, xq in (("xh", xh_full), ("xl", xl_full)):
            xT_e = np.zeros((128, 8, C), dtype=xq.dtype)
            # xT_e[p, d, :ne] = xq[toks, d*128+p].T
            xT_e[:, :, :ne] = xq[toks].T.reshape(8, 128, ne).transpose(1, 0, 2)
            xs[name] = xT_e
        g_e = np.zeros((1, C), dtype=np.float32)
        g_e[0, :ne] = gates[toks, e] / (S1 * S2)
        in_maps.append({
            **xs,
            "w1": _pack_w(W1[e], S1),
            "w2": _pack_w(W2[e], S2),
            "bb": np.ascontiguousarray(np.concatenate(
                [(S1 * b1[e]).reshape(8, 128).T,
                 (S1 * S2 * b2[e]).reshape(8, 128).T], axis=1)),
            "go": np.concatenate(
                [g_e, np.ones((1, 128), dtype=np.float32)], axis=1),
        })
    return in_maps


def kernel(x, W1, b1, W2, b2, Wg, bg):
    from concourse import bass_utils

    x = np.ascontiguousarray(np.asarray(x, dtype=np.float32))
    W1 = np.asarray(W1, dtype=np.float32)
    b1 = np.asarray(b1, dtype=np.float32)
    W2 = np.asarray(W2, dtype=np.float32)
    b2 = np.asarray(b2, dtype=np.float32)
    Wg = np.asarray(Wg, dtype=np.float32)
    bg = np.asarray(bg, dtype=np.float32)
    n = x.shape[0]

    gates, order = _route(x, Wg, bg)
    tok_lists = [np.where((order == e).any(axis=1))[0] for e in range(NUM_EXPERTS)]
    max_load = max(len(t) for t in tok_lists)
    C, tok_tiles = _plan_tiles(max_load)

    key = (C, tuple(tok_tiles))
    if key not in _prog_cache:
        _prog_cache[key] = _build_program((C, tok_tiles))
    nc = _prog_cache[key]

    in_maps = _make_in_maps(x, W1, b1, W2, b2, gates, order, tok_lists, C)
    res = bass_utils.run_bass_kernel_spmd(nc, in_maps, list(range(NUM_EXPERTS)))
    # yT result: [128, 8, C] bf16 -> y_e[c, o*128+p] = yT[p, o, c]
    yT_all = np.stack([res.results[e]["yT"].astype(np.float32)
                       for e in range(NUM_EXPERTS)])

    # scatter-add the two expert contributions per token (already gated)
    slot = np.zeros((NUM_EXPERTS, n), dtype=np.int64)
    for e in range(NUM_EXPERTS):
        slot[e, tok_lists[e]] = np.arange(len(tok_lists[e]))
    rows = np.arange(n)
    out = np.zeros((n, D), dtype=np.float32)
    for k in range(TOP_K):
        ek = order[:, k]
        picked = yT_all[ek, :, :, slot[ek, rows]]   # [n, 128, 8]
        out += picked.transpose(0, 2, 1).reshape(n, D)
    return out


# revision 9
# speedup vs baseline: 1.3990x; 1.0183x over previous
"""Trainium2 Bass kernel for an 8-expert top-2 MoE layer.

Strategy (expert-parallel, per the sharding hint): the host computes the
tiny gating matmul + softmax + top-2 routing, gathers each expert's
assigned tokens, and ships one expert per NeuronCore. Each core runs the
heavy 2-layer MLP for its expert over its assigned tokens, applies the
gate weights on-device, and the host scatter-adds the two expert
contributions per token.

The MLP matmuls run as fp8(e4m3) DoubleRow pair-matmuls (each
instruction contracts K=256 = 2 k-tiles at half-rate-per-row), with
*residual compensation* to keep accuracy: every operand A is shipped as
a hi/lo pair (A_hi = fp8(A), A_lo = fp8(A - A_hi), same scale), and each
1024-contraction runs three streams

    A_hi @ W_hi  +  A_lo @ W_hi  +  A_hi @ W_lo

which costs 12 pair-matmuls per 128-wide output group (vs 8 full-rate
matmuls for f32r) -> 0.75 cycles/row/layer equivalent, and leaves only
residual-of-residual error (~3e-3 max-rel, gate is 2e-2).

Scales are powers of two folded into host-prepped constants:
  W1 is shipped as fp8(64*W1), so PSUM1 = 64*(x@W1);
  h is evicted as relu(PSUM1 + 64*b1) = 64*h (max ~206 < 240 = e4m3 max)
  via one ACT relu (bias AP), then cast to fp8 (hi) on ACT and the
  residual (lo) computed on DVE;
  W2 is shipped as fp8(128*W2), so PSUM2 = 8192*(h@W2), and the y
  eviction folds b2*8192 and gate/8192 into one (psum + b2') * gate'
  DVE op; y ships bf16.

Schedule: token tiles (<=512, one fp32 PSUM bank) are software-
pipelined as L1(0) L1(1) L2(0) L1(2) L2(1) ... so the h-eviction chain
(ACT relu -> ACT fp8 cast -> DVE residual) of tile t hides under tile
t+1's layer-1 matmuls. x/y use a tile-major DRAM layout (each tile's 8
d-rows contiguous per partition -> >=2KB DMA runs at full model
bandwidth, 128 descriptors per transfer). The first tile is ~296 tokens
so its x lands early but its groups still consume weight strips no
faster than the (HWDGE-serialized) strips arrive. Warm-up matmuls off a
memset tile keep the PE p-state ramping from ~1us with no DMA
dependency; the last tile's output DMA is split in halves so only a
quarter of it trails the final matmul.
"""

import numpy as np

NUM_EXPERTS = 8
TOP_K = 2
D = 1024
S1 = 64.0     # W1/h scale
S2 = 128.0    # W2 scale (gate folds 1/(S1*S2))

_prog_cache = {}


def _plan_tiles(max_load):
    """Token-tile sizes covering max_load: a ~296-token first tile (early
    x arrival without starving on weight strips), then 512s, then a tail
    rounded to a multiple of 8 (fp32r gate matmul ISA restriction)."""
    r8 = lambda v: -(-v // 8) * 8
    if max_load <= 296:
        tiles = [r8(max(max_load, 8))]
    else:
        n512, rem = divmod(max_load - 296, 512)
        tiles = [296] + [512] * n512
        if rem:
            tiles.append(r8(rem))
    return sum(tiles), tiles


def _build_program(tile_plan):
    """Build the per-core Bass program: one expert's MLP over C tokens."""
    from contextlib import ExitStack

    import concourse.tile as tile
    from concourse import bacc, mybir

    f32 = mybir.dt.float32
    f32r = mybir.dt.float32r
    f8 = mybir.dt.float8e4
    bf16 = mybir.dt.bfloat16
    DR = mybir.MatmulPerfMode.DoubleRow
    ADD = mybir.AluOpType.add
    MULT = mybir.AluOpType.mult
    RELU = mybir.ActivationFunctionType.Relu
    COPY = mybir.ActivationFunctionType.Copy

    C, tok_tiles = tile_plan

    nc = bacc.Bacc("TRN2", target_bir_lowering=False, debug=False,
                   num_devices=NUM_EXPERTS)

    # host-packed layouts (see _make_in_maps), all e4m3 except consts:
    #   xh/xl: [128, 8*C] tile-major: cols [8*pos_t + d*TT_t + c]
    #          = q(x_gathered[pos_t + c, d*128 + p]) hi/lo
    #   w1:  [8, 128, 2, 8, 128]  w1[j, p, v, d, r] = q(64*W1[d*128+p, j*128+r])
    #   w2:  [8, 128, 2, 8, 128]  w2[o, p, v, j, r] = q(128*W2[j*128+p, o*128+r])
    #   bb:  [128, 16] f32        [64*b1 | 8192*b2] per-partition
    #   go:  [1, C+128] f32r      [gate row / 8192 | ones row]
    #   yT:  [128, 8*C] bf16      tile-major like xh/xl, gated y
    xh_d = nc.dram_tensor("xh", [128, 8 * C], f8, kind="ExternalInput").ap()
    xl_d = nc.dram_tensor("xl", [128, 8 * C], f8, kind="ExternalInput").ap()
    w1_d = nc.dram_tensor("w1", [8, 128, 2, 8, 128], f8, kind="ExternalInput").ap()
    w2_d = nc.dram_tensor("w2", [8, 128, 2, 8, 128], f8, kind="ExternalInput").ap()
    bb_d = nc.dram_tensor("bb", [128, 16], f32, kind="ExternalInput").ap()
    go_d = nc.dram_tensor("go", [1, C + 128], f32r, kind="ExternalInput").ap()
    yT_d = nc.dram_tensor("yT", [128, 8 * C], bf16, kind="ExternalOutput").ap()

    with tile.TileContext(nc) as tc, ExitStack() as ctx:
        wpool = ctx.enter_context(tc.tile_pool(name="w", bufs=1))
        cpool = ctx.enter_context(tc.tile_pool(name="const", bufs=1))
        xpool = ctx.enter_context(tc.tile_pool(name="x", bufs=2))
        hxpool = ctx.enter_context(tc.tile_pool(name="hx", bufs=3))
        hpool = ctx.enter_context(tc.tile_pool(name="h", bufs=2))
        ypool = ctx.enter_context(tc.tile_pool(name="y", bufs=2))
        gpool = ctx.enter_context(tc.tile_pool(name="g", bufs=2))
        php = ctx.enter_context(tc.tile_pool(name="ph", bufs=3, space="PSUM"))
        pyp = ctx.enter_context(tc.tile_pool(name="py", bufs=3, space="PSUM"))
        pgp = ctx.enter_context(tc.tile_pool(name="pg", bufs=2, space="PSUM"))

        # PE warm-up fed by a small memset (no DMA dependency): dummy bf16
        # matmuls keep the PE busy from ~1us so the cost-model p-state
        # reaches full speed right as the first real matmuls arrive.
        wsrc = cpool.tile([1, 512], bf16, tag="wsrc")
        nc.vector.memset(wsrc[:], 1.0)
        warm = pgp.tile([128, 512], f32, tag="gps")
        for _ in range(9):
            nc.tensor.matmul(warm[:], wsrc[:, 0:128], wsrc[:, 0:512],
                             start=True, stop=True)

        # DMA emission in consumption order (transfers serialize on the
        # DMA bus and dispatches on HWDGE at ~650ns each): w1 strip 0 and
        # tile-0 x first, consts, remaining w1 strips, tile-1 x, w2 strips
        TT0 = tok_tiles[0]
        w1_sb = [None] * 8
        w1_first = wpool.tile([128, 2, 8, 128], f8, tag="w1_0")
        nc.sync.dma_start(w1_first[:], w1_d[0])
        w1_sb[0] = w1_first
        xh0 = xpool.tile([128, 8, TT0], f8, tag="xh")
        nc.sync.dma_start(xh0[:], xh_d[:, 0:8 * TT0])
        xl0 = xpool.tile([128, 8, TT0], f8, tag="xl")
        nc.sync.dma_start(xl0[:], xl_d[:, 0:8 * TT0])
        bb_sb = cpool.tile([128, 16], f32, tag="bb")
        nc.sync.dma_start(bb_sb[:], bb_d[:])
        go_sb = cpool.tile([1, C + 128], f32r, tag="go")
        nc.sync.dma_start(go_sb[:], go_d[:])
        g_sb = go_sb[:, 0:C]
        ones_sb = go_sb[:, C:C + 128]
        for j in range(1, 8):
            w1_strip = wpool.tile([128, 2, 8, 128], f8, tag=f"w1_{j}")
            nc.sync.dma_start(w1_strip[:], w1_d[j])
            w1_sb[j] = w1_strip

        x_tiles = [None] * len(tok_tiles)
        x_tiles[0] = (xh0, xl0)
        if len(tok_tiles) > 1:
            TT1 = tok_tiles[1]
            sl1 = slice(8 * TT0, 8 * (TT0 + TT1))
            xh1 = xpool.tile([128, 8, TT1], f8, tag="xh")
            nc.sync.dma_start(xh1[:], xh_d[:, sl1])
            xl1 = xpool.tile([128, 8, TT1], f8, tag="xl")
            nc.sync.dma_start(xl1[:], xl_d[:, sl1])
            x_tiles[1] = (xh1, xl1)
        w2_sb = [None] * 8
        for o in range(8):
            w2_strip = wpool.tile([128, 2, 8, 128], f8, tag=f"w2_{o}")
            nc.sync.dma_start(w2_strip[:], w2_d[o])
            w2_sb[o] = w2_strip

        tile_pos = np.cumsum([0] + tok_tiles).tolist()
        ntile = len(tok_tiles)
        h_tiles = [None] * ntile
        g_tiles = [None] * ntile

        def emit_l1(t):
            """Layer 1 + gate broadcast of tile t; leaves h8/hl8 + g_bc."""
            TT = tok_tiles[t]

            # prefetch x for tile t+1 (tiles 0 and 1 issued upfront)
            nt = t + 1
            if nt < ntile and x_tiles[nt] is None:
                TTn = tok_tiles[nt]
                nsl = slice(8 * tile_pos[nt], 8 * (tile_pos[nt] + TTn))
                xh_p = xpool.tile([128, 8, TTn], f8, tag="xh")
                nc.sync.dma_start(xh_p[:], xh_d[:, nsl])
                xl_p = xpool.tile([128, 8, TTn], f8, tag="xl")
                nc.sync.dma_start(xl_p[:], xl_d[:, nsl])
                x_tiles[nt] = (xh_p, xl_p)

            xh_sb, xl_sb = x_tiles[t]

            # layer 1: 64*h^T[j] = relu(64*sum_d W1[d,j]^T x^T[d] + 64*b1[j])
            # 3 fp8 DoubleRow streams: xh@W1h + xl@W1h + xh@W1l
            h8 = [hpool.tile([128, 2, TT], f8, tag=f"h8_{q}", name=f"h8_{q}")
                  for q in range(4)]
            hl8 = [hpool.tile([128, 2, TT], f8, tag=f"hl8_{q}", name=f"hl8_{q}")
                   for q in range(4)]
            for j in range(8):
                ph = php.tile([128, TT], f32, tag="ph")
                n = 0
                for v, xs in ((0, xh_sb), (0, xl_sb), (1, xh_sb)):
                    for q in range(4):
                        nc.tensor.matmul(ph[:],
                                         w1_sb[j][:, v, 2 * q:2 * q + 2, :],
                                         xs[:, 2 * q:2 * q + 2, :],
                                         start=(n == 0), stop=(n == 11),
                                         perf_mode=DR)
                        n += 1
                hx32 = hxpool.tile([128, TT], f32, tag="hx32")
                nc.scalar.activation(hx32[:], ph[:], RELU,
                                     bias=bb_sb[:, j:j + 1], scale=1.0)
                h8s = h8[j // 2][:, j % 2, :]
                nc.scalar.activation(h8s, hx32[:], COPY)
                nc.vector.scalar_tensor_tensor(hl8[j // 2][:, j % 2, :],
                                               h8s, -1.0, hx32[:],
                                               op0=MULT, op1=ADD)
            h_tiles[t] = (h8, hl8)

            # broadcast gate row across partitions: G[p, n] = g[n]
            tsl = slice(tile_pos[t], tile_pos[t] + TT)
            g_ps = pgp.tile([128, TT], f32, tag="gps")
            nc.tensor.matmul(g_ps[:], ones_sb[:], g_sb[:, tsl],
                             start=True, stop=True)
            g_bc = gpool.tile([128, TT], f32, tag="gbc")
            nc.vector.tensor_copy(g_bc[:], g_ps[:])
            g_tiles[t] = g_bc

        def emit_l2(t):
            """Layer 2 + gate + output DMA of tile t."""
            TT = tok_tiles[t]
            h8, hl8 = h_tiles[t]
            g_bc = g_tiles[t]
            last = t == ntile - 1

            # layer 2 + gate: y^T[o] = (sum_j W2[j,o]^T h^T[j] + b2[o]) * g
            # 3 fp8 DoubleRow streams: h8@W2h + hl8@W2h + h8@W2l
            ybig = ypool.tile([128, 8, TT], bf16, tag="y")
            for o in range(8):
                py = pyp.tile([128, TT], f32, tag="py")
                n = 0
                for v, hs in ((0, h8), (0, hl8), (1, h8)):
                    for q in range(4):
                        nc.tensor.matmul(py[:],
                                         w2_sb[o][:, v, 2 * q:2 * q + 2, :],
                                         hs[q][:],
                                         start=(n == 0), stop=(n == 11),
                                         perf_mode=DR)
                        n += 1
                nc.vector.scalar_tensor_tensor(ybig[:, o, :], py[:],
                                               bb_sb[:, 8 + o:9 + o],
                                               g_bc[:], op0=ADD, op1=MULT)
                if last and o == 3:
                    # dispatch the first half early so only a quarter of
                    # the final output DMA trails the last matmul
                    nc.sync.dma_start(
                        yT_d[:, 8 * tile_pos[t]:8 * tile_pos[t] + 4 * TT],
                        ybig[:, 0:4, :])
            base = 8 * tile_pos[t]
            if last:
                nc.sync.dma_start(yT_d[:, base + 4 * TT:base + 8 * TT],
                                  ybig[:, 4:8, :])
            else:
                nc.sync.dma_start(yT_d[:, base:base + 8 * TT], ybig[:])

        # software pipeline: layer 1 of tile t+1 runs (on PE) before layer
        # 2 of tile t, so the h-eviction chain (ACT relu -> ACT fp8 cast ->
        # DVE residual) of tile t hides under tile t+1's layer-1 matmuls.
        emit_l1(0)
        for t in range(1, ntile):
            emit_l1(t)
            emit_l2(t - 1)
        emit_l2(ntile - 1)

    nc.compile()
    return nc


def _route(x, Wg, bg):
    """Host gating: fp32 softmax + top-2, matching jax.lax.top_k semantics."""
    logits = x @ Wg + bg
    m = logits.max(axis=1, keepdims=True)
    e = np.exp(logits - m)
    gates = e / e.sum(axis=1, keepdims=True)
    # stable argsort on negated values = ties broken by lower index (jax)
    order = np.argsort(-gates, axis=1, kind="stable")[:, :TOP_K]
    return gates, order


def _q8(a):
    import ml_dtypes
    return np.asarray(a).astype(ml_dtypes.float8_e4m3)


def _pack_w(W, scale):
    """[1024,1024] -> [8, 128, 2, 8, 128] hi/lo fp8 strips.

    out[s, p, v, d, r] = q_v(scale * W[d*128+p, s*128+r])
    """
    Ws = (W * scale).astype(np.float32)
    Wh = _q8(Ws)
    Wl = _q8(Ws - Wh.astype(np.float32))
    packs = []
    for Wv in (Wh, Wl):
        # [d, p, s, r] -> [s, p, d, r]
        packs.append(Wv.reshape(8, 128, 8, 128).transpose(2, 1, 0, 3))
    # -> [s, p, v, d, r]
    return np.ascontiguousarray(np.stack(packs, axis=2))


def _pack_x_tiles(xq, toks, tok_tiles, C):
    """Gather + transpose + tile-major pack: [128, 8*C] fp8."""
    out = np.zeros((128, 8 * C), dtype=xq.dtype)
    ne = len(toks)
    pos = 0
    for TT in tok_tiles:
        take = toks[pos:pos + TT]
        if len(take):
            # [nt, 1024] -> [128, 8, nt]
            seg = xq[take].T.reshape(8, 128, len(take)).transpose(1, 0, 2)
            blk = out[:, 8 * pos:8 * (pos + TT)].reshape(128, 8, TT)
            blk[:, :, :len(take)] = seg
        pos += TT
    return out


def _make_in_maps(x, W1, b1, W2, b2, gates, order, tok_lists, C, tok_tiles):
    xh_full = _q8(x)
    xl_full = _q8(x - xh_full.astype(np.float32))
    in_maps = []
    for e in range(NUM_EXPERTS):
        toks = tok_lists[e]
        g_e = np.zeros((1, C), dtype=np.float32)
        g_e[0, :len(toks)] = gates[toks, e] / (S1 * S2)
        in_maps.append({
            "xh": _pack_x_tiles(xh_full, toks, tok_tiles, C),
            "xl": _pack_x_tiles(xl_full, toks, tok_tiles, C),
            "w1": _pack_w(W1[e], S1),
            "w2": _pack_w(W2[e], S2),
            "bb": np.ascontiguousarray(np.concatenate(
                [(S1 * b1[e]).reshape(8, 128).T,
                 (S1 * S2 * b2[e]).reshape(8, 128).T], axis=1)),
            "go": np.concatenate(
                [g_e, np.ones((1, 128), dtype=np.float32)], axis=1),
        })
    return in_maps


def kernel(x, W1, b1, W2, b2, Wg, bg):
    from concourse import bass_utils

    x = np.ascontiguousarray(np.asarray(x, dtype=np.float32))
    W1 = np.asarray(W1, dtype=np.float32)
    b1 = np.asarray(b1, dtype=np.float32)
    W2 = np.asarray(W2, dtype=np.float32)
    b2 = np.asarray(b2, dtype=np.float32)
    Wg = np.asarray(Wg, dtype=np.float32)
    bg = np.asarray(bg, dtype=np.float32)
    n = x.shape[0]

    gates, order = _route(x, Wg, bg)
    tok_lists = [np.where((order == e).any(axis=1))[0] for e in range(NUM_EXPERTS)]
    max_load = max(len(t) for t in tok_lists)
    C, tok_tiles = _plan_tiles(max_load)

    key = (C, tuple(tok_tiles))
    if key not in _prog_cache:
        _prog_cache[key] = _build_program((C, tok_tiles))
    nc = _prog_cache[key]

    in_maps = _make_in_maps(x, W1, b1, W2, b2, gates, order, tok_lists, C,
                            tok_tiles)
    res = bass_utils.run_bass_kernel_spmd(nc, in_maps, list(range(NUM_EXPERTS)))
    # yT result: tile-major [128, 8*C] bf16 -> [E, 128, 8, C] f32
    yT_all = np.empty((NUM_EXPERTS, 128, 8, C), dtype=np.float32)
    for e in range(NUM_EXPERTS):
        flat = res.results[e]["yT"].astype(np.float32)
        pos = 0
        for TT in tok_tiles:
            yT_all[e, :, :, pos:pos + TT] = (
                flat[:, 8 * pos:8 * (pos + TT)].reshape(128, 8, TT))
            pos += TT

    # scatter-add the two expert contributions per token (already gated)
    slot = np.zeros((NUM_EXPERTS, n), dtype=np.int64)
    for e in range(NUM_EXPERTS):
        slot[e, tok_lists[e]] = np.arange(len(tok_lists[e]))
    rows = np.arange(n)
    out = np.zeros((n, D), dtype=np.float32)
    for k in range(TOP_K):
        ek = order[:, k]
        picked = yT_all[ek, :, :, slot[ek, rows]]   # [n, 128, 8]
        out += picked.transpose(0, 2, 1).reshape(n, D)
    return out


# revision 13
# speedup vs baseline: 1.4100x; 1.0079x over previous
"""Trainium2 Bass kernel for an 8-expert top-2 MoE layer.

Strategy (expert-parallel, per the sharding hint): the host computes the
tiny gating matmul + softmax + top-2 routing, gathers each expert's
assigned tokens, and ships one expert per NeuronCore. Each core runs the
heavy 2-layer MLP for its expert over its assigned tokens, applies the
gate weights on-device, and the host scatter-adds the two expert
contributions per token.

The MLP matmuls run as fp8(e4m3) DoubleRow pair-matmuls (each
instruction contracts K=256 = 2 k-tiles at half-rate-per-row), with
*residual compensation* to keep accuracy: every operand A is shipped as
a hi/lo pair (A_hi = fp8(A), A_lo = fp8(A - A_hi), same scale), and each
1024-contraction runs three streams

    A_hi @ W_hi  +  A_lo @ W_hi  +  A_hi @ W_lo

which costs 12 pair-matmuls per 128-wide output group (vs 8 full-rate
matmuls for f32r) -> 0.75 cycles/row/layer equivalent, and leaves only
residual-of-residual error (~3e-3 max-rel, gate is 2e-2).

Scales are powers of two folded into host-prepped constants:
  W1 is shipped as fp8(64*W1), so PSUM1 = 64*(x@W1);
  h is evicted as relu(PSUM1 + 64*b1) = 64*h (max ~206 < 240 = e4m3 max)
  via one ACT relu (bias AP), then cast to fp8 (hi) on ACT and the
  residual (lo) computed on DVE;
  W2 is shipped as fp8(128*W2), so PSUM2 = 8192*(h@W2), and the y
  eviction folds b2*8192 and gate/8192 into one (psum + b2') * gate'
  DVE op; y ships bf16.

Schedule: token tiles (<=512, one fp32 PSUM bank) are software-
pipelined as L1(0) L1(1) L2(0) L1(2) L2(1) ... so the h-eviction chain
(ACT relu -> ACT fp8 cast -> DVE residual) of tile t hides under tile
t+1's layer-1 matmuls. x/y use a tile-major DRAM layout (each tile's 8
d-rows contiguous per partition -> >=2KB DMA runs at full model
bandwidth, 128 descriptors per transfer). The first tile is ~296 tokens
so its x lands early but its groups still consume weight strips no
faster than the (HWDGE-serialized) strips arrive. Warm-up matmuls off a
memset tile keep the PE p-state ramping from ~1us with no DMA
dependency; the last tile's output DMA is split in halves so only a
quarter of it trails the final matmul.
"""

import numpy as np

NUM_EXPERTS = 8
TOP_K = 2
D = 1024
S1 = 64.0     # W1/h scale
S2 = 128.0    # W2 scale (gate folds 1/(S1*S2))

_prog_cache = {}


def _plan_tiles(max_load):
    """Token-tile sizes covering max_load: a ~296-token first tile (early
    x arrival without starving on weight strips), then 512s, then a tail
    rounded to a multiple of 8 (fp32r gate matmul ISA restriction)."""
    r8 = lambda v: -(-v // 8) * 8
    if max_load <= 296:
        tiles = [r8(max(max_load, 8))]
    else:
        n512, rem = divmod(max_load - 296, 512)
        tiles = [296] + [512] * n512
        if rem:
            tiles.append(r8(rem))
    return sum(tiles), tiles


def _build_program(tile_plan):
    """Build the per-core Bass program: one expert's MLP over C tokens."""
    from contextlib import ExitStack

    import concourse.tile as tile
    from concourse import bacc, mybir

    f32 = mybir.dt.float32
    f32r = mybir.dt.float32r
    f8 = mybir.dt.float8e4
    bf16 = mybir.dt.bfloat16
    DR = mybir.MatmulPerfMode.DoubleRow
    ADD = mybir.AluOpType.add
    MULT = mybir.AluOpType.mult
    RELU = mybir.ActivationFunctionType.Relu
    COPY = mybir.ActivationFunctionType.Copy

    C, tok_tiles = tile_plan

    nc = bacc.Bacc("TRN2", target_bir_lowering=False, debug=False,
                   num_devices=NUM_EXPERTS)

    # host-packed layouts (see _make_in_maps), all e4m3 except consts:
    #   xh/xl: [128, 8*C] tile-major: cols [8*pos_t + d*TT_t + c]
    #          = q(x_gathered[pos_t + c, d*128 + p]) hi/lo
    #   w1:  [8, 128, 2, 8, 128]  w1[j, p, v, d, r] = q(64*W1[d*128+p, j*128+r])
    #   w2:  [8, 128, 2, 8, 128]  w2[o, p, v, j, r] = q(128*W2[j*128+p, o*128+r])
    #   bb:  [128, 16] f32        [64*b1 | 8192*b2] per-partition
    #   go:  [1, C+128] f32r      [gate row / 8192 | ones row]
    #   yT:  [128, 8*C] bf16      tile-major like xh/xl, gated y
    xh_d = nc.dram_tensor("xh", [128, 8 * C], f8, kind="ExternalInput").ap()
    xl_d = nc.dram_tensor("xl", [128, 8 * C], f8, kind="ExternalInput").ap()
    w1_d = nc.dram_tensor("w1", [8, 128, 2, 8, 128], f8, kind="ExternalInput").ap()
    w2_d = nc.dram_tensor("w2", [8, 128, 2, 8, 128], f8, kind="ExternalInput").ap()
    bb_d = nc.dram_tensor("bb", [128, 16], f32, kind="ExternalInput").ap()
    go_d = nc.dram_tensor("go", [1, C + 128], f32r, kind="ExternalInput").ap()
    yT_d = nc.dram_tensor("yT", [128, 8 * C], bf16, kind="ExternalOutput").ap()

    with tile.TileContext(nc) as tc, ExitStack() as ctx:
        wpool = ctx.enter_context(tc.tile_pool(name="w", bufs=1))
        cpool = ctx.enter_context(tc.tile_pool(name="const", bufs=1))
        xpool = ctx.enter_context(tc.tile_pool(name="x", bufs=2))
        hxpool = ctx.enter_context(tc.tile_pool(name="hx", bufs=3))
        hpool = ctx.enter_context(tc.tile_pool(name="h", bufs=2))
        ypool = ctx.enter_context(tc.tile_pool(name="y", bufs=2))
        gpool = ctx.enter_context(tc.tile_pool(name="g", bufs=2))
        php = ctx.enter_context(tc.tile_pool(name="ph", bufs=4, space="PSUM"))
        pyp = ctx.enter_context(tc.tile_pool(name="py", bufs=3, space="PSUM"))
        pgp = ctx.enter_context(tc.tile_pool(name="pg", bufs=1, space="PSUM"))

        # PE warm-up fed by a small memset (no DMA dependency): dummy bf16
        # matmuls keep the PE busy from ~1us so the cost-model p-state
        # reaches full speed right as the first real matmuls arrive. A
        # dummy relu warms the ACT function table (1.3us load) in the
        # shadow of the DMA ramp.
        wsrc = cpool.tile([1, 512], bf16, tag="wsrc")
        nc.vector.memset(wsrc[:], 1.0)
        dummy = cpool.tile([1, 128], bf16, tag="dummy")
        nc.scalar.activation(dummy[:], wsrc[0:1, 0:128],
                             mybir.ActivationFunctionType.Relu,
                             bias=wsrc[0:1, 0:1], scale=1.0)
        warm = pgp.tile([128, 512], f32, tag="gps")
        for _ in range(9):
            nc.tensor.matmul(warm[:], wsrc[:, 0:128], wsrc[:, 0:512],
                             start=True, stop=True)

        # DMA emission in consumption order (transfers serialize on the
        # DMA bus and dispatches on HWDGE at ~650ns each): w1 strip 0 and
        # tile-0 x first, consts, remaining w1 strips, tile-1 x, w2 strips
        TT0 = tok_tiles[0]
        w1_sb = [None] * 8
        w1_first = wpool.tile([128, 2, 8, 128], f8, tag="w1_0")
        nc.sync.dma_start(w1_first[:], w1_d[0])
        w1_sb[0] = w1_first
        xh0 = xpool.tile([128, 8, TT0], f8, tag="xh")
        nc.sync.dma_start(xh0[:], xh_d[:, 0:8 * TT0])
        xl0 = xpool.tile([128, 8, TT0], f8, tag="xl")
        nc.sync.dma_start(xl0[:], xl_d[:, 0:8 * TT0])
        bb_sb = cpool.tile([128, 16], f32, tag="bb")
        nc.sync.dma_start(bb_sb[:], bb_d[:])
        go_sb = cpool.tile([1, C + 128], f32r, tag="go")
        nc.sync.dma_start(go_sb[:], go_d[:])
        g_sb = go_sb[:, 0:C]
        ones_sb = go_sb[:, C:C + 128]
        for j in range(1, 8):
            w1_strip = wpool.tile([128, 2, 8, 128], f8, tag=f"w1_{j}")
            nc.sync.dma_start(w1_strip[:], w1_d[j])
            w1_sb[j] = w1_strip

        x_tiles = [None] * len(tok_tiles)
        x_tiles[0] = (xh0, xl0)
        if len(tok_tiles) > 1:
            TT1 = tok_tiles[1]
            sl1 = slice(8 * TT0, 8 * (TT0 + TT1))
            xh1 = xpool.tile([128, 8, TT1], f8, tag="xh")
            nc.sync.dma_start(xh1[:], xh_d[:, sl1])
            xl1 = xpool.tile([128, 8, TT1], f8, tag="xl")
            nc.sync.dma_start(xl1[:], xl_d[:, sl1])
            x_tiles[1] = (xh1, xl1)
        w2_sb = [None] * 8
        for o in range(8):
            w2_strip = wpool.tile([128, 2, 8, 128], f8, tag=f"w2_{o}")
            nc.sync.dma_start(w2_strip[:], w2_d[o])
            w2_sb[o] = w2_strip

        tile_pos = np.cumsum([0] + tok_tiles).tolist()
        ntile = len(tok_tiles)
        h_tiles = [None] * ntile

        def emit_l1(t):
            """Layer 1 + gate broadcast of tile t; leaves h8/hl8 + g_bc."""
            TT = tok_tiles[t]

            # prefetch x for tile t+1 (tiles 0 and 1 issued upfront)
            nt = t + 1
            if nt < ntile and x_tiles[nt] is None:
                TTn = tok_tiles[nt]
                nsl = slice(8 * tile_pos[nt], 8 * (tile_pos[nt] + TTn))
                xh_p = xpool.tile([128, 8, TTn], f8, tag="xh")
                nc.sync.dma_start(xh_p[:], xh_d[:, nsl])
                xl_p = xpool.tile([128, 8, TTn], f8, tag="xl")
                nc.sync.dma_start(xl_p[:], xl_d[:, nsl])
                x_tiles[nt] = (xh_p, xl_p)

            xh_sb, xl_sb = x_tiles[t]

            # layer 1: 64*h^T[j] = relu(64*sum_d W1[d,j]^T x^T[d] + 64*b1[j])
            # 3 fp8 DoubleRow streams: xh@W1h + xl@W1h + xh@W1l
            h8 = [hpool.tile([128, 2, TT], f8, tag=f"h8_{q}", name=f"h8_{q}")
                  for q in range(4)]
            hl8 = [hpool.tile([128, 2, TT], f8, tag=f"hl8_{q}", name=f"hl8_{q}")
                   for q in range(4)]
            for j in range(8):
                ph = php.tile([128, TT], f32, tag="ph")
                n = 0
                for v, xs in ((0, xh_sb), (0, xl_sb), (1, xh_sb)):
                    for q in range(4):
                        nc.tensor.matmul(ph[:],
                                         w1_sb[j][:, v, 2 * q:2 * q + 2, :],
                                         xs[:, 2 * q:2 * q + 2, :],
                                         start=(n == 0), stop=(n == 11),
                                         perf_mode=DR)
                        n += 1
                hx32 = hxpool.tile([128, TT], f32, tag="hx32")
                nc.scalar.activation(hx32[:], ph[:], RELU,
                                     bias=bb_sb[:, j:j + 1], scale=1.0)
                h8s = h8[j // 2][:, j % 2, :]
                nc.scalar.activation(h8s, hx32[:], COPY)
                nc.vector.scalar_tensor_tensor(hl8[j // 2][:, j % 2, :],
                                               h8s, -1.0, hx32[:],
                                               op0=MULT, op1=ADD)
            h_tiles[t] = (h8, hl8)

        def emit_l2(t):
            """Gate broadcast + layer 2 + output DMA of tile t."""
            TT = tok_tiles[t]
            h8, hl8 = h_tiles[t]
            last = t == ntile - 1

            # broadcast gate row across partitions: G[p, n] = g[n]
            tsl = slice(tile_pos[t], tile_pos[t] + TT)
            g_ps = pgp.tile([128, TT], f32, tag="gps")
            nc.tensor.matmul(g_ps[:], ones_sb[:], g_sb[:, tsl],
                             start=True, stop=True)
            g_bc = gpool.tile([128, TT], f32, tag="gbc")
            nc.vector.tensor_copy(g_bc[:], g_ps[:])

            # layer 2 + gate: y^T[o] = (sum_j W2[j,o]^T h^T[j] + b2[o]) * g
            # 3 fp8 DoubleRow streams: h8@W2h + hl8@W2h + h8@W2l
            ybig = ypool.tile([128, 8, TT], bf16, tag="y")
            for o in range(8):
                py = pyp.tile([128, TT], f32, tag="py")
                n = 0
                for v, hs in ((0, h8), (0, hl8), (1, h8)):
                    for q in range(4):
                        nc.tensor.matmul(py[:],
                                         w2_sb[o][:, v, 2 * q:2 * q + 2, :],
                                         hs[q][:],
                                         start=(n == 0), stop=(n == 11),
                                         perf_mode=DR)
                        n += 1
                nc.vector.scalar_tensor_tensor(ybig[:, o, :], py[:],
                                               bb_sb[:, 8 + o:9 + o],
                                               g_bc[:], op0=ADD, op1=MULT)
                if last and o == 3:
                    # dispatch early pieces so only one o-group of the
                    # final output DMA trails the last matmul
                    nc.sync.dma_start(
                        yT_d[:, 8 * tile_pos[t]:8 * tile_pos[t] + 4 * TT],
                        ybig[:, 0:4, :])
                if last and o == 6:
                    nc.sync.dma_start(
                        yT_d[:, 8 * tile_pos[t] + 4 * TT:
                             8 * tile_pos[t] + 7 * TT],
                        ybig[:, 4:7, :])
            base = 8 * tile_pos[t]
            if last:
                nc.sync.dma_start(yT_d[:, base + 7 * TT:base + 8 * TT],
                                  ybig[:, 7:8, :])
            else:
                nc.sync.dma_start(yT_d[:, base:base + 8 * TT], ybig[:])

        # software pipeline: layer 1 of tile t+1 runs (on PE) before layer
        # 2 of tile t, so the h-eviction chain (ACT relu -> ACT fp8 cast ->
        # DVE residual) of tile t hides under tile t+1's layer-1 matmuls.
        emit_l1(0)
        for t in range(1, ntile):
            emit_l1(t)
            emit_l2(t - 1)
        emit_l2(ntile - 1)

    nc.compile()
    return nc


def _route(x, Wg, bg):
    """Host gating: fp32 softmax + top-2, matching jax.lax.top_k semantics."""
    logits = x @ Wg + bg
    m = logits.max(axis=1, keepdims=True)
    e = np.exp(logits - m)
    gates = e / e.sum(axis=1, keepdims=True)
    # stable argsort on negated values = ties broken by lower index (jax)
    order = np.argsort(-gates, axis=1, kind="stable")[:, :TOP_K]
    return gates, order


def _q8(a):
    import ml_dtypes
    return np.asarray(a).astype(ml_dtypes.float8_e4m3)


def _pack_w(W, scale):
    """[1024,1024] -> [8, 128, 2, 8, 128] hi/lo fp8 strips.

    out[s, p, v, d, r] = q_v(scale * W[d*128+p, s*128+r])
    """
    Ws = (W * scale).astype(np.float32)
    Wh = _q8(Ws)
    Wl = _q8(Ws - Wh.astype(np.float32))
    packs = []
    for Wv in (Wh, Wl):
        # [d, p, s, r] -> [s, p, d, r]
        packs.append(Wv.reshape(8, 128, 8, 128).transpose(2, 1, 0, 3))
    # -> [s, p, v, d, r]
    return np.ascontiguousarray(np.stack(packs, axis=2))


def _pack_x_tiles(xq, toks, tok_tiles, C):
    """Gather + transpose + tile-major pack: [128, 8*C] fp8."""
    out = np.zeros((128, 8 * C), dtype=xq.dtype)
    ne = len(toks)
    pos = 0
    for TT in tok_tiles:
        take = toks[pos:pos + TT]
        if len(take):
            # [nt, 1024] -> [128, 8, nt]
            seg = xq[take].T.reshape(8, 128, len(take)).transpose(1, 0, 2)
            blk = out[:, 8 * pos:8 * (pos + TT)].reshape(128, 8, TT)
            blk[:, :, :len(take)] = seg
        pos += TT
    return out


def _make_in_maps(x, W1, b1, W2, b2, gates, order, tok_lists, C, tok_tiles):
    xh_full = _q8(x)
    xl_full = _q8(x - xh_full.astype(np.float32))
    in_maps = []
    for e in range(NUM_EXPERTS):
        toks = tok_lists[e]
        g_e = np.zeros((1, C), dtype=np.float32)
        g_e[0, :len(toks)] = gates[toks, e] / (S1 * S2)
        in_maps.append({
            "xh": _pack_x_tiles(xh_full, toks, tok_tiles, C),
            "xl": _pack_x_tiles(xl_full, toks, tok_tiles, C),
            "w1": _pack_w(W1[e], S1),
            "w2": _pack_w(W2[e], S2),
            "bb": np.ascontiguousarray(np.concatenate(
                [(S1 * b1[e]).reshape(8, 128).T,
                 (S1 * S2 * b2[e]).reshape(8, 128).T], axis=1)),
            "go": np.concatenate(
                [g_e, np.ones((1, 128), dtype=np.float32)], axis=1),
        })
    return in_maps


def kernel(x, W1, b1, W2, b2, Wg, bg):
    from concourse import bass_utils

    x = np.ascontiguousarray(np.asarray(x, dtype=np.float32))
    W1 = np.asarray(W1, dtype=np.float32)
    b1 = np.asarray(b1, dtype=np.float32)
    W2 = np.asarray(W2, dtype=np.float32)
    b2 = np.asarray(b2, dtype=np.float32)
    Wg = np.asarray(Wg, dtype=np.float32)
    bg = np.asarray(bg, dtype=np.float32)
    n = x.shape[0]

    gates, order = _route(x, Wg, bg)
    tok_lists = [np.where((order == e).any(axis=1))[0] for e in range(NUM_EXPERTS)]
    max_load = max(len(t) for t in tok_lists)
    C, tok_tiles = _plan_tiles(max_load)

    key = (C, tuple(tok_tiles))
    if key not in _prog_cache:
        _prog_cache[key] = _build_program((C, tok_tiles))
    nc = _prog_cache[key]

    in_maps = _make_in_maps(x, W1, b1, W2, b2, gates, order, tok_lists, C,
                            tok_tiles)
    res = bass_utils.run_bass_kernel_spmd(nc, in_maps, list(range(NUM_EXPERTS)))
    # yT result: tile-major [128, 8*C] bf16 -> [E, 128, 8, C] f32
    yT_all = np.empty((NUM_EXPERTS, 128, 8, C), dtype=np.float32)
    for e in range(NUM_EXPERTS):
        flat = res.results[e]["yT"].astype(np.float32)
        pos = 0
        for TT in tok_tiles:
            yT_all[e, :, :, pos:pos + TT] = (
                flat[:, 8 * pos:8 * (pos + TT)].reshape(128, 8, TT))
            pos += TT

    # scatter-add the two expert contributions per token (already gated)
    slot = np.zeros((NUM_EXPERTS, n), dtype=np.int64)
    for e in range(NUM_EXPERTS):
        slot[e, tok_lists[e]] = np.arange(len(tok_lists[e]))
    rows = np.arange(n)
    out = np.zeros((n, D), dtype=np.float32)
    for k in range(TOP_K):
        ek = order[:, k]
        picked = yT_all[ek, :, :, slot[ek, rows]]   # [n, 128, 8]
        out += picked.transpose(0, 2, 1).reshape(n, D)
    return out


# revision 15
# speedup vs baseline: 1.4419x; 1.0226x over previous
"""Trainium2 Bass kernel for an 8-expert top-2 MoE layer.

Strategy (expert-parallel, per the sharding hint): the host computes the
tiny gating matmul + softmax + top-2 routing, gathers each expert's
assigned tokens, and ships one expert per NeuronCore. Each core runs the
heavy 2-layer MLP for its expert over its assigned tokens, applies the
gate weights on-device, and the host scatter-adds the two expert
contributions per token.

The MLP matmuls run as fp8(e4m3) DoubleRow pair-matmuls (each
instruction contracts K=256 = 2 k-tiles at half-rate-per-row), with
*residual compensation* to keep accuracy: every operand A is shipped as
a hi/lo pair (A_hi = fp8(A), A_lo = fp8(A - A_hi), same scale), and each
1024-contraction runs three streams

    A_hi @ W_hi  +  A_lo @ W_hi  +  A_hi @ W_lo

which costs 12 pair-matmuls per 128-wide output group (vs 8 full-rate
matmuls for f32r) -> 0.75 cycles/row/layer equivalent, and leaves only
residual-of-residual error (~3e-3 max-rel, gate is 2e-2).

Scales are powers of two folded into host-prepped constants:
  W1 is shipped as fp8(64*W1), so PSUM1 = 64*(x@W1);
  h is evicted as relu(PSUM1 + 64*b1) = 64*h (max ~206 < 240 = e4m3 max)
  via one ACT relu (bias AP), then cast to fp8 (hi) on ACT and the
  residual (lo) computed on DVE;
  W2 is shipped as fp8(128*W2), so PSUM2 = 8192*(h@W2), and the y
  eviction folds b2*8192 and gate/8192 into one (psum + b2') * gate'
  DVE op; y ships bf16.

Schedule: token tiles (<=512, one fp32 PSUM bank) are software-
pipelined as L1(0) L1(1) L2(0) L1(2) L2(1) ... so the h-eviction chain
(ACT relu -> ACT fp8 cast -> DVE residual) of tile t hides under tile
t+1's layer-1 matmuls. x/y use a tile-major DRAM layout (each tile's 8
d-rows contiguous per partition -> >=2KB DMA runs at full model
bandwidth, 128 descriptors per transfer). The first tile is ~296 tokens
so its x lands early but its groups still consume weight strips no
faster than the (HWDGE-serialized) strips arrive. Warm-up matmuls off a
memset tile keep the PE p-state ramping from ~1us with no DMA
dependency; the last tile's output DMA is split in halves so only a
quarter of it trails the final matmul.
"""

import numpy as np

NUM_EXPERTS = 8
TOP_K = 2
D = 1024
S1 = 64.0     # W1/h scale
S2 = 128.0    # W2 scale (gate folds 1/(S1*S2))

_prog_cache = {}


def _plan_tiles(max_load):
    """Token-tile sizes covering max_load: a ~296-token first tile (early
    x arrival without starving on weight strips), then 512s, then a tail
    rounded to a multiple of 8 (fp32r gate matmul ISA restriction)."""
    r8 = lambda v: -(-v // 8) * 8
    if max_load <= 296:
        tiles = [r8(max(max_load, 8))]
    else:
        n512, rem = divmod(max_load - 296, 512)
        tiles = [296] + [512] * n512
        if rem:
            tiles.append(r8(rem))
    return sum(tiles), tiles


def _build_program(tile_plan):
    """Build the per-core Bass program: one expert's MLP over C tokens."""
    from contextlib import ExitStack

    import concourse.tile as tile
    from concourse import bacc, mybir

    f32 = mybir.dt.float32
    f32r = mybir.dt.float32r
    f8 = mybir.dt.float8e4
    bf16 = mybir.dt.bfloat16
    DR = mybir.MatmulPerfMode.DoubleRow
    ADD = mybir.AluOpType.add
    MULT = mybir.AluOpType.mult
    RELU = mybir.ActivationFunctionType.Relu
    COPY = mybir.ActivationFunctionType.Copy

    C, tok_tiles = tile_plan

    nc = bacc.Bacc("TRN2", target_bir_lowering=False, debug=False,
                   num_devices=NUM_EXPERTS)

    # host-packed layouts (see _make_in_maps), all e4m3 except consts:
    #   xh/xl: [128, 8*C] tile-major: cols [8*pos_t + d*TT_t + c]
    #          = q(x_gathered[pos_t + c, d*128 + p]) hi/lo
    #   w1:  [8, 128, 2, 8, 128]  w1[j, p, v, d, r] = q(64*W1[d*128+p, j*128+r])
    #   w2:  [8, 128, 2, 8, 128]  w2[o, p, v, j, r] = q(128*W2[j*128+p, o*128+r])
    #   bb:  [128, 16] f32        [64*b1 | 8192*b2] per-partition
    #   go:  [1, C+128] f32r      [gate row / 8192 | ones row]
    #   yT:  [128, 8*C] bf16      tile-major like xh/xl, gated y
    xh_d = nc.dram_tensor("xh", [128, 8 * C], f8, kind="ExternalInput").ap()
    xl_d = nc.dram_tensor("xl", [128, 8 * C], f8, kind="ExternalInput").ap()
    w1_d = nc.dram_tensor("w1", [8, 128, 2, 8, 128], f8, kind="ExternalInput").ap()
    w2_d = nc.dram_tensor("w2", [8, 128, 2, 8, 128], f8, kind="ExternalInput").ap()
    bb_d = nc.dram_tensor("bb", [128, 16], f32, kind="ExternalInput").ap()
    gb_d = nc.dram_tensor("gb", [128, C], f32, kind="ExternalInput").ap()
    yT_d = nc.dram_tensor("yT", [128, 8 * C], bf16, kind="ExternalOutput").ap()

    with tile.TileContext(nc) as tc, ExitStack() as ctx:
        wpool = ctx.enter_context(tc.tile_pool(name="w", bufs=1))
        cpool = ctx.enter_context(tc.tile_pool(name="const", bufs=1))
        xpool = ctx.enter_context(tc.tile_pool(name="x", bufs=2))
        hxpool = ctx.enter_context(tc.tile_pool(name="hx", bufs=3))
        hpool = ctx.enter_context(tc.tile_pool(name="h", bufs=2))
        ypool = ctx.enter_context(tc.tile_pool(name="y", bufs=2))
        gpool = ctx.enter_context(tc.tile_pool(name="g", bufs=2))
        php = ctx.enter_context(tc.tile_pool(name="ph", bufs=4, space="PSUM"))
        pyp = ctx.enter_context(tc.tile_pool(name="py", bufs=3, space="PSUM"))
        pwp = ctx.enter_context(tc.tile_pool(name="pw", bufs=1, space="PSUM"))

        # PE warm-up fed by a small memset (no DMA dependency): dummy bf16
        # matmuls keep the PE busy from ~1us so the cost-model p-state
        # reaches full speed right as the first real matmuls arrive. A
        # dummy relu warms the ACT function table (1.3us load) in the
        # shadow of the DMA ramp.
        wsrc = cpool.tile([1, 256], bf16, tag="wsrc")
        nc.vector.memset(wsrc[:], 1.0)
        dummy = cpool.tile([1, 128], bf16, tag="dummy")
        nc.scalar.activation(dummy[:], wsrc[0:1, 0:128],
                             mybir.ActivationFunctionType.Relu,
                             bias=wsrc[0:1, 0:1], scale=1.0)
        warm = pwp.tile([128, 256], f32, tag="warm")
        for _ in range(20):
            nc.tensor.matmul(warm[:], wsrc[:, 0:128], wsrc[:, 0:256],
                             start=True, stop=True)

        # DMA emission in consumption order (transfers serialize on the
        # DMA bus and dispatches on HWDGE at ~650ns each): w1 strip 0 and
        # tile-0 x first, consts, remaining w1 strips, tile-1 x, w2 strips
        TT0 = tok_tiles[0]
        w1_sb = [None] * 8
        w1_first = wpool.tile([128, 2, 8, 128], f8, tag="w1_0")
        nc.sync.dma_start(w1_first[:], w1_d[0])
        w1_sb[0] = w1_first
        xh0 = xpool.tile([128, 8, TT0], f8, tag="xh")
        nc.sync.dma_start(xh0[:], xh_d[:, 0:8 * TT0])
        xl0 = xpool.tile([128, 8, TT0], f8, tag="xl")
        nc.sync.dma_start(xl0[:], xl_d[:, 0:8 * TT0])
        bb_sb = cpool.tile([128, 16], f32, tag="bb")
        nc.sync.dma_start(bb_sb[:], bb_d[:])
        for j in range(1, 8):
            w1_strip = wpool.tile([128, 2, 8, 128], f8, tag=f"w1_{j}")
            nc.sync.dma_start(w1_strip[:], w1_d[j])
            w1_sb[j] = w1_strip

        g_tiles = [None] * len(tok_tiles)
        gb0 = gpool.tile([128, TT0], f32, tag="gbc")
        nc.sync.dma_start(gb0[:], gb_d[:, 0:TT0])
        g_tiles[0] = gb0
        x_tiles = [None] * len(tok_tiles)
        x_tiles[0] = (xh0, xl0)
        if len(tok_tiles) > 1:
            TT1 = tok_tiles[1]
            sl1 = slice(8 * TT0, 8 * (TT0 + TT1))
            xh1 = xpool.tile([128, 8, TT1], f8, tag="xh")
            nc.sync.dma_start(xh1[:], xh_d[:, sl1])
            xl1 = xpool.tile([128, 8, TT1], f8, tag="xl")
            nc.sync.dma_start(xl1[:], xl_d[:, sl1])
            x_tiles[1] = (xh1, xl1)
            gb1 = gpool.tile([128, TT1], f32, tag="gbc")
            nc.sync.dma_start(gb1[:], gb_d[:, TT0:TT0 + TT1])
            g_tiles[1] = gb1
        w2_sb = [None] * 8
        for o in range(8):
            w2_strip = wpool.tile([128, 2, 8, 128], f8, tag=f"w2_{o}")
            nc.sync.dma_start(w2_strip[:], w2_d[o])
            w2_sb[o] = w2_strip

        tile_pos = np.cumsum([0] + tok_tiles).tolist()
        ntile = len(tok_tiles)
        h_tiles = [None] * ntile
        assert len(g_tiles) == ntile

        def emit_l1(t):
            """Layer 1 + gate broadcast of tile t; leaves h8/hl8 + g_bc."""
            TT = tok_tiles[t]

            # prefetch x for tile t+1 (tiles 0 and 1 issued upfront)
            nt = t + 1
            if nt < ntile and x_tiles[nt] is None:
                TTn = tok_tiles[nt]
                nsl = slice(8 * tile_pos[nt], 8 * (tile_pos[nt] + TTn))
                xh_p = xpool.tile([128, 8, TTn], f8, tag="xh")
                nc.sync.dma_start(xh_p[:], xh_d[:, nsl])
                xl_p = xpool.tile([128, 8, TTn], f8, tag="xl")
                nc.sync.dma_start(xl_p[:], xl_d[:, nsl])
                x_tiles[nt] = (xh_p, xl_p)
                gb_p = gpool.tile([128, TTn], f32, tag="gbc")
                nc.sync.dma_start(gb_p[:],
                                  gb_d[:, tile_pos[nt]:tile_pos[nt] + TTn])
                g_tiles[nt] = gb_p

            xh_sb, xl_sb = x_tiles[t]

            # layer 1: 64*h^T[j] = relu(64*sum_d W1[d,j]^T x^T[d] + 64*b1[j])
            # 3 fp8 DoubleRow streams: xh@W1h + xl@W1h + xh@W1l
            h8 = [hpool.tile([128, 2, TT], f8, tag=f"h8_{q}", name=f"h8_{q}")
                  for q in range(4)]
            hl8 = [hpool.tile([128, 2, TT], f8, tag=f"hl8_{q}", name=f"hl8_{q}")
                   for q in range(4)]
            for j in range(8):
                ph = php.tile([128, TT], f32, tag="ph")
                n = 0
                for v, xs in ((0, xh_sb), (0, xl_sb), (1, xh_sb)):
                    for q in range(4):
                        nc.tensor.matmul(ph[:],
                                         w1_sb[j][:, v, 2 * q:2 * q + 2, :],
                                         xs[:, 2 * q:2 * q + 2, :],
                                         start=(n == 0), stop=(n == 11),
                                         perf_mode=DR)
                        n += 1
                hx32 = hxpool.tile([128, TT], f32, tag="hx32")
                nc.scalar.activation(hx32[:], ph[:], RELU,
                                     bias=bb_sb[:, j:j + 1], scale=1.0)
                h8s = h8[j // 2][:, j % 2, :]
                nc.scalar.activation(h8s, hx32[:], COPY)
                nc.vector.scalar_tensor_tensor(hl8[j // 2][:, j % 2, :],
                                               h8s, -1.0, hx32[:],
                                               op0=MULT, op1=ADD)
            h_tiles[t] = (h8, hl8)

        def emit_l2(t):
            """Gate broadcast + layer 2 + output DMA of tile t."""
            TT = tok_tiles[t]
            h8, hl8 = h_tiles[t]
            last = t == ntile - 1

            g_bc = g_tiles[t]

            # layer 2 + gate: y^T[o] = (sum_j W2[j,o]^T h^T[j] + b2[o]) * g
            # 3 fp8 DoubleRow streams: h8@W2h + hl8@W2h + h8@W2l
            ybig = ypool.tile([128, 8, TT], bf16, tag="y")
            for o in range(8):
                py = pyp.tile([128, TT], f32, tag="py")
                n = 0
                for v, hs in ((0, h8), (0, hl8), (1, h8)):
                    for q in range(4):
                        nc.tensor.matmul(py[:],
                                         w2_sb[o][:, v, 2 * q:2 * q + 2, :],
                                         hs[q][:],
                                         start=(n == 0), stop=(n == 11),
                                         perf_mode=DR)
                        n += 1
                nc.vector.scalar_tensor_tensor(ybig[:, o, :], py[:],
                                               bb_sb[:, 8 + o:9 + o],
                                               g_bc[:], op0=ADD, op1=MULT)
                if last and o == 3:
                    # dispatch early pieces so only one o-group of the
                    # final output DMA trails the last matmul
                    nc.sync.dma_start(
                        yT_d[:, 8 * tile_pos[t]:8 * tile_pos[t] + 4 * TT],
                        ybig[:, 0:4, :])
                if last and o == 6:
                    nc.sync.dma_start(
                        yT_d[:, 8 * tile_pos[t] + 4 * TT:
                             8 * tile_pos[t] + 7 * TT],
                        ybig[:, 4:7, :])
            base = 8 * tile_pos[t]
            if last:
                nc.sync.dma_start(yT_d[:, base + 7 * TT:base + 8 * TT],
                                  ybig[:, 7:8, :])
            else:
                nc.sync.dma_start(yT_d[:, base:base + 8 * TT], ybig[:])

        # software pipeline: layer 1 of tile t+1 runs (on PE) before layer
        # 2 of tile t, so the h-eviction chain (ACT relu -> ACT fp8 cast ->
        # DVE residual) of tile t hides under tile t+1's layer-1 matmuls.
        emit_l1(0)
        for t in range(1, ntile):
            emit_l1(t)
            emit_l2(t - 1)
        emit_l2(ntile - 1)

    nc.compile()
    return nc


def _route(x, Wg, bg):
    """Host gating: fp32 softmax + top-2, matching jax.lax.top_k semantics."""
    logits = x @ Wg + bg
    m = logits.max(axis=1, keepdims=True)
    e = np.exp(logits - m)
    gates = e / e.sum(axis=1, keepdims=True)
    # stable argsort on negated values = ties broken by lower index (jax)
    order = np.argsort(-gates, axis=1, kind="stable")[:, :TOP_K]
    return gates, order


def _q8(a):
    import ml_dtypes
    return np.asarray(a).astype(ml_dtypes.float8_e4m3)


def _pack_w(W, scale):
    """[1024,1024] -> [8, 128, 2, 8, 128] hi/lo fp8 strips.

    out[s, p, v, d, r] = q_v(scale * W[d*128+p, s*128+r])
    """
    Ws = (W * scale).astype(np.float32)
    Wh = _q8(Ws)
    Wl = _q8(Ws - Wh.astype(np.float32))
    packs = []
    for Wv in (Wh, Wl):
        # [d, p, s, r] -> [s, p, d, r]
        packs.append(Wv.reshape(8, 128, 8, 128).transpose(2, 1, 0, 3))
    # -> [s, p, v, d, r]
    return np.ascontiguousarray(np.stack(packs, axis=2))


def _pack_x_tiles(xq, toks, tok_tiles, C):
    """Gather + transpose + tile-major pack: [128, 8*C] fp8."""
    out = np.zeros((128, 8 * C), dtype=xq.dtype)
    ne = len(toks)
    pos = 0
    for TT in tok_tiles:
        take = toks[pos:pos + TT]
        if len(take):
            # [nt, 1024] -> [128, 8, nt]
            seg = xq[take].T.reshape(8, 128, len(take)).transpose(1, 0, 2)
            blk = out[:, 8 * pos:8 * (pos + TT)].reshape(128, 8, TT)
            blk[:, :, :len(take)] = seg
        pos += TT
    return out


def _make_in_maps(x, W1, b1, W2, b2, gates, order, tok_lists, C, tok_tiles):
    xh_full = _q8(x)
    xl_full = _q8(x - xh_full.astype(np.float32))
    in_maps = []
    for e in range(NUM_EXPERTS):
        toks = tok_lists[e]
        g_e = np.zeros(C, dtype=np.float32)
        g_e[:len(toks)] = gates[toks, e] / (S1 * S2)
        in_maps.append({
            "xh": _pack_x_tiles(xh_full, toks, tok_tiles, C),
            "xl": _pack_x_tiles(xl_full, toks, tok_tiles, C),
            "w1": _pack_w(W1[e], S1),
            "w2": _pack_w(W2[e], S2),
            "bb": np.ascontiguousarray(np.concatenate(
                [(S1 * b1[e]).reshape(8, 128).T,
                 (S1 * S2 * b2[e]).reshape(8, 128).T], axis=1)),
            "gb": np.ascontiguousarray(
                np.broadcast_to(g_e, (128, C))),
        })
    return in_maps


def kernel(x, W1, b1, W2, b2, Wg, bg):
    from concourse import bass_utils

    x = np.ascontiguousarray(np.asarray(x, dtype=np.float32))
    W1 = np.asarray(W1, dtype=np.float32)
    b1 = np.asarray(b1, dtype=np.float32)
    W2 = np.asarray(W2, dtype=np.float32)
    b2 = np.asarray(b2, dtype=np.float32)
    Wg = np.asarray(Wg, dtype=np.float32)
    bg = np.asarray(bg, dtype=np.float32)
    n = x.shape[0]

    gates, order = _route(x, Wg, bg)
    tok_lists = [np.where((order == e).any(axis=1))[0] for e in range(NUM_EXPERTS)]
    max_load = max(len(t) for t in tok_lists)
    C, tok_tiles = _plan_tiles(max_load)

    key = (C, tuple(tok_tiles))
    if key not in _prog_cache:
        _prog_cache[key] = _build_program((C, tok_tiles))
    nc = _prog_cache[key]

    in_maps = _make_in_maps(x, W1, b1, W2, b2, gates, order, tok_lists, C,
                            tok_tiles)
    res = bass_utils.run_bass_kernel_spmd(nc, in_maps, list(range(NUM_EXPERTS)))
    # yT result: tile-major [128, 8*C] bf16 -> [E, 128, 8, C] f32
    yT_all = np.empty((NUM_EXPERTS, 128, 8, C), dtype=np.float32)
    for e in range(NUM_EXPERTS):
        flat = res.results[e]["yT"].astype(np.float32)
        pos = 0
        for TT in tok_tiles:
            yT_all[e, :, :, pos:pos + TT] = (
                flat[:, 8 * pos:8 * (pos + TT)].reshape(128, 8, TT))
            pos += TT

    # scatter-add the two expert contributions per token (already gated)
    slot = np.zeros((NUM_EXPERTS, n), dtype=np.int64)
    for e in range(NUM_EXPERTS):
        slot[e, tok_lists[e]] = np.arange(len(tok_lists[e]))
    rows = np.arange(n)
    out = np.zeros((n, D), dtype=np.float32)
    for k in range(TOP_K):
        ek = order[:, k]
        picked = yT_all[ek, :, :, slot[ek, rows]]   # [n, 128, 8]
        out += picked.transpose(0, 2, 1).reshape(n, D)
    return out
